# revision 19
# baseline (speedup 1.0000x reference)
"""Trainium2 Bass kernel for the sparse_attention (channel-attention) module.

Computation per sample (x_s, xh_s are [512, 1152] slices):
    theta = Wt @ x_s  + bt        (fold 1/512 into Wt, bt)
    phi   = Wp @ xh_s + bp
    g     = Wg @ xh_s + bg
    att   = theta @ phi^T         (contract over n; includes the /512)
    Wa    = (Ww*inv) @ att        (BN scale folded into Ww; 512^3 GEMM,
                                   cheaper than y = att @ g at 512^2*1152)
    out   = Wa @ g + xf           (xf = x + BN offset, bf16, host-folded)

Sharding: pure data parallel, 4 samples per core across 8 cores.

All GEMMs run in fp8 (e4m3, max 240) with DoubleRow perf mode. Per-tensor
scales come from a host sample-0 forward estimate with margin, so the
compiled program is data-independent. theta/phi are produced transposed
(thetaT[n,i]); att is produced as att[i,j] (stationary thetaT) which is the
stationary operand Wa needs, and WaT[j,o] is the stationary operand of the
out matmuls.

Schedule notes (from trace analysis):
- DMA engines are shared round-robin across ACTIVE rings, so bulk prefetch
  must queue BEHIND head-critical transfers. Inputs ride the sync HW ring
  in need-order (then bulk xf/next-sample FIFO behind); weights + out
  writes ride the scalar HW ring. The slow-start gpsimd SW ring is unused.
- PSUM: tag "pd" (DVE-evicted: theta/phi/out) and "pa" (ACT: g/att/Wa) get
  2 two-bank slots each so slot reuse always waits on the matching engine,
  with >=1.7us reuse distance in the emission order below.
- out evictions alternate DVE / Pool(gpsimd) so the out phase is not
  DVE-serial-bound; Wa sits between theta/phi p3 and p4 of the next
  sample's thpg so its ACT evictions hide under DVE-evicted fills.
"""

import numpy as np
import ml_dtypes

import concourse.bass as bass
import concourse.mybir as mybir
from concourse import bacc
from concourse.tile import TileContext
from concourse import bass_utils

B, DIM, H, W = 32, 512, 48, 24
N = H * W            # 1152
P = 128
CB = DIM // P        # 4 channel blocks
NB = N // P          # 9 n blocks
NCH = 3
CHW = N // NCH       # 384
NCORES = 8
BL = B // NCORES     # 4 samples per core

_f32 = mybir.dt.float32
_bf16 = mybir.dt.bfloat16
_fp8 = mybir.dt.float8e4
_add = mybir.AluOpType.add
_mult = mybir.AluOpType.mult
_DR = mybir.MatmulPerfMode.DoubleRow
_IDENT = mybir.ActivationFunctionType.Identity

FP8NP = ml_dtypes.float8_e4m3
BF16NP = ml_dtypes.bfloat16
FP8TGT = 192.0                      # of 240 max: saturation headroom

_PROGRAM = None


def _build_program():
    nc = bacc.Bacc("TRN2", target_bir_lowering=False, debug=False)

    # x and x_h fp8, chunk-interleaved: xin[s][:, c, 0] = x chunk c,
    # xin[s][:, c, 1] = x_h chunk c.
    xin = nc.dram_tensor("xin", [BL, P, NCH, 2, CB, CHW], _fp8,
                         kind="ExternalInput").ap()
    xf = nc.dram_tensor("xf", [BL, P, NCH, CB, CHW], _bf16,
                        kind="ExternalInput").ap()
    wall = nc.dram_tensor("wall", [P, 4, CB, DIM], _fp8,
                          kind="ExternalInput").ap()
    # bias rows (pre-scaled to consumer fp8 grid): a single-partition 2KB
    # tensor, broadcast across partitions on-device via a 1-contraction
    # matmul (saves 0.5MB of head-critical DMA)
    brow = nc.dram_tensor("brow", [1, 2, DIM], _bf16,
                          kind="ExternalInput").ap()
    # per-partition columns: bg*s_g per o-block (0:4), eviction scales (4:10)
    ccols = nc.dram_tensor("ccols", [P, 16], _f32, kind="ExternalInput").ap()
    out4 = nc.dram_tensor("out4", [BL, P, NCH, CB, CHW], _bf16,
                          kind="ExternalOutput").ap()

    with TileContext(nc) as tc:
        with tc.tile_pool(name="const", bufs=1) as cpool, \
             tc.tile_pool(name="xin", bufs=3) as xpool, \
             tc.tile_pool(name="xfin", bufs=3) as xfpool, \
             tc.tile_pool(name="work", bufs=6) as wpool, \
             tc.tile_pool(name="attwa", bufs=4) as apool, \
             tc.tile_pool(name="out", bufs=2) as opool, \
             tc.tile_pool(name="psum", bufs=2, space="PSUM") as psum:

            ccols_sb = cpool.tile([P, 16], _f32, tag="ccols")
            cb2 = cpool.tile([P, 2, 2, DIM], _bf16, tag="cbias")
            row_sb = cpool.tile([P, 2, DIM], _bf16, tag="brow")
            ones_sb = cpool.tile([P, P], _bf16, tag="ones")
            w_sb = cpool.tile([P, 4, CB, DIM], _fp8, tag="wall")
            wt_sb = w_sb[:, 0]
            wp_sb = w_sb[:, 1]
            wg_sb = w_sb[:, 2]
            ww_sb = w_sb[:, 3]

            btb2 = cb2[:, 0]               # [P, 2, DIM]
            btb = cb2[:, 0, 0]             # [P, DIM]
            bpb2 = cb2[:, 1]
            bpb = cb2[:, 1, 0]
            bgc = [ccols_sb[:, i:i + 1] for i in range(0, 4)]
            c_theta = ccols_sb[:, 4:5]
            c_phi = ccols_sb[:, 5:6]
            c_att = ccols_sb[:, 6:7]
            c_g = ccols_sb[:, 7:8]
            c_wa = ccols_sb[:, 8:9]
            c_out = ccols_sb[:, 9:10]

            st = [dict() for _ in range(BL)]

            def pd_tile():
                return psum.tile([P, 2, DIM], _f32, tag="pd", name="pd")

            def pa_tile():
                return psum.tile([P, 2, DIM], _f32, tag="pa", name="pa")

            def emit_warmup(n, nbias):
                """Dummy matmuls on a zeroed tile: keeps the PE continuously
                busy from the preamble until real data lands, so the p-state
                ramp (0.65 -> 1.2 -> 2.4GHz after 3us continuous) completes
                before the first real matmul. After `nbias` dummies, the PE
                broadcasts the bias rows (partition 0 of row_sb) across all
                partitions into a psum tile; ACT copies them into the
                duplicated SBUF layout the paired theta/phi evictions use."""
                zt = cpool.tile([P, 2, CHW], _fp8, tag="warm")
                nc.vector.memset(zt, 0)
                nc.vector.memset(ones_sb, 1.0)
                wps = pa_tile()
                bps = pd_tile()
                for i in range(n):
                    nc.tensor.matmul(wps[:, i % 2, :CHW], zt[:, :, 0:P], zt,
                                     start=True, stop=True, perf_mode=_DR)
                    if i == nbias:
                        for j in range(2):
                            nc.tensor.matmul(bps[:, j], ones_sb[0:1],
                                             row_sb[0:1, j],
                                             start=True, stop=True)
                        nc.scalar.activation(cb2[:, :, 0], bps, _IDENT,
                                             bias=0.0, scale=1.0)
                        nc.scalar.activation(cb2[:, :, 1], bps, _IDENT,
                                             bias=0.0, scale=1.0)

            def dma_head():
                """Both HW rings, need-order; bulk FIFOs behind."""
                x_sb = xpool.tile([P, NCH, 2, CB, CHW], _fp8, tag="xin",
                                  name="x_sb")
                xf_sb = xfpool.tile([P, NCH, CB, CHW], _bf16, tag="xf",
                                    name="xf_sb")
                st[0].update(x_sb=x_sb, xf_sb=xf_sb)
                x1 = xpool.tile([P, NCH, 2, CB, CHW], _fp8, tag="xin",
                                name="x_sb")
                xf1 = xfpool.tile([P, NCH, CB, CHW], _bf16, tag="xf",
                                  name="xf_sb")
                st[1].update(x_sb=x1, xf_sb=xf1)
                # sync HW ring (fast): consts, then x/xh halves in use
                # order, then the bulk residuals FIFO behind
                nc.sync.dma_start(ccols_sb, ccols)
                nc.sync.dma_start(row_sb[0:1], brow)
                nc.sync.dma_start(x_sb[:, 0, 0], xin[0][:, 0, 0])
                nc.sync.dma_start(x_sb[:, 0, 1], xin[0][:, 0, 1])
                nc.sync.dma_start(x_sb[:, 1, 0], xin[0][:, 1, 0])
                nc.sync.dma_start(x_sb[:, 1, 1], xin[0][:, 1, 1])
                nc.sync.dma_start(x_sb[:, 2, 0], xin[0][:, 2, 0])
                nc.sync.dma_start(x_sb[:, 2, 1], xin[0][:, 2, 1])
                nc.sync.dma_start(xf_sb, xf[0])
                nc.sync.dma_start(xf1, xf[1])
                # scalar ring (parallel): weights in two fat-line (4KB)
                # transfers
                nc.scalar.dma_start(w_sb[:, 0:2], wall[:, 0:2])
                nc.scalar.dma_start(w_sb[:, 2:4], wall[:, 2:4])
                # slow-start software ring: next-sample bulk prefetch
                nc.gpsimd.dma_start(x1, xin[1])

            def dma_sample(s):
                x_sb = xpool.tile([P, NCH, 2, CB, CHW], _fp8, tag="xin",
                                  name="x_sb")
                xf_sb = xfpool.tile([P, NCH, CB, CHW], _bf16, tag="xf",
                                    name="xf_sb")
                nc.gpsimd.dma_start(x_sb, xin[s])
                nc.sync.dma_start(xf_sb, xf[s])
                st[s].update(x_sb=x_sb, xf_sb=xf_sb)

            def alloc_work(s):
                d = st[s]
                d["thetaT"] = wpool.tile([P, NB, DIM], _fp8, tag="work",
                                         name="thetaT")
                d["phiT"] = wpool.tile([P, NB, DIM], _fp8, tag="work",
                                       name="phiT")
                d["g_sb"] = wpool.tile([P, NCH, CB, CHW], _fp8, tag="work",
                                       name="g_sb")

            def conv_pair(x_sb, half, w, nbs):
                ps2 = pd_tile()
                for j, nb in enumerate(nbs):
                    c, jj = divmod(nb, NCH)
                    for k in range(CB // 2):
                        nc.tensor.matmul(
                            ps2[:, j],
                            x_sb[:, c, half, 2 * k:2 * k + 2,
                                 jj * P:(jj + 1) * P],
                            w[:, 2 * k:2 * k + 2],
                            start=(k == 0), stop=(k == CB // 2 - 1),
                            perf_mode=_DR)
                return ps2

            def emit_theta(s, p):
                d = st[s]
                nbs = [2 * p, 2 * p + 1] if 2 * p + 1 < NB else [NB - 1]
                ps2 = conv_pair(d["x_sb"], 0, wt_sb, nbs)
                if len(nbs) == 2:
                    nc.vector.scalar_tensor_tensor(
                        d["thetaT"][:, 2 * p:2 * p + 2], ps2, c_theta, btb2,
                        _mult, _add)
                else:
                    nc.vector.scalar_tensor_tensor(
                        d["thetaT"][:, NB - 1], ps2[:, 0], c_theta, btb,
                        _mult, _add)

            def emit_phi(s, p):
                d = st[s]
                nbs = [2 * p, 2 * p + 1] if 2 * p + 1 < NB else [NB - 1]
                ps2 = conv_pair(d["x_sb"], 1, wp_sb, nbs)
                if len(nbs) == 2:
                    nc.vector.scalar_tensor_tensor(
                        d["phiT"][:, 2 * p:2 * p + 2], ps2, c_phi, bpb2,
                        _mult, _add)
                else:
                    nc.vector.scalar_tensor_tensor(
                        d["phiT"][:, NB - 1], ps2[:, 0], c_phi, bpb,
                        _mult, _add)

            def emit_g(s, ob, chs):
                d = st[s]
                ps2 = pa_tile()
                for j, ch in enumerate(chs):
                    for k in range(CB // 2):
                        nc.tensor.matmul(
                            ps2[:, j, :CHW],
                            wg_sb[:, 2 * k:2 * k + 2, ob * P:(ob + 1) * P],
                            d["x_sb"][:, ch, 1, 2 * k:2 * k + 2, :],
                            start=(k == 0), stop=(k == CB // 2 - 1),
                            perf_mode=_DR)
                if len(chs) == 2:
                    nc.scalar.activation(
                        d["g_sb"][:, 0:2, ob], ps2[:, :, :CHW], _IDENT,
                        bias=bgc[ob], scale=c_g)
                else:
                    nc.scalar.activation(
                        d["g_sb"][:, chs[0], ob], ps2[:, 0, :CHW], _IDENT,
                        bias=bgc[ob], scale=c_g)

            def emit_thpg_A(s):
                alloc_work(s)
                for p in range(3):
                    emit_theta(s, p)
                    emit_phi(s, p)
                    emit_g(s, p, [0, 1])
                    emit_g(s, p, [2])

            def emit_att(s, tail=False):
                """att[i,j] = c_att * thetaT^T @ phiT (stationary thetaT)."""
                d = st[s]
                thetaT, phiT = d["thetaT"], d["phiT"]
                att_sb = apool.tile([P, CB, DIM], _fp8, tag="attwa",
                                    name="att_sb")
                d["att_sb"] = att_sb
                for p in range(CB // 2):
                    ps2 = pa_tile()
                    for j in range(2):
                        ib = 2 * p + j
                        for k in range(NB // 2):
                            nc.tensor.matmul(
                                ps2[:, j],
                                thetaT[:, 2 * k:2 * k + 2,
                                       ib * P:(ib + 1) * P],
                                phiT[:, 2 * k:2 * k + 2],
                                start=(k == 0), stop=False, perf_mode=_DR)
                        nc.tensor.matmul(
                            ps2[:, j], thetaT[:, NB - 1, ib * P:(ib + 1) * P],
                            phiT[:, NB - 1], start=False, stop=True)
                        if tail:
                            if j == 0:
                                nc.vector.tensor_scalar_mul(
                                    att_sb[:, 2 * p], ps2[:, 0], c_att)
                            else:
                                nc.scalar.activation(
                                    att_sb[:, 2 * p + 1], ps2[:, 1], _IDENT,
                                    bias=0.0, scale=c_att)
                    if not tail:
                        nc.scalar.activation(att_sb[:, 2 * p:2 * p + 2], ps2,
                                             _IDENT, bias=0.0, scale=c_att)

            def emit_wa(s, tail=False):
                """WaT[j,o] = c_wa * att^T @ (Ww*inv) (stationary att)."""
                d = st[s]
                att_sb = d["att_sb"]
                wa_sb = apool.tile([P, CB, DIM], _fp8, tag="attwa",
                                   name="wa_sb")
                d["wa_sb"] = wa_sb
                for p in range(CB // 2):
                    ps2 = pa_tile()
                    for j in range(2):
                        jb = 2 * p + j
                        for k in range(CB // 2):
                            nc.tensor.matmul(
                                ps2[:, j],
                                att_sb[:, 2 * k:2 * k + 2,
                                       jb * P:(jb + 1) * P],
                                ww_sb[:, 2 * k:2 * k + 2],
                                start=(k == 0), stop=(k == CB // 2 - 1),
                                perf_mode=_DR)
                        if tail:
                            if j == 0:
                                nc.vector.tensor_scalar_mul(
                                    wa_sb[:, 2 * p], ps2[:, 0], c_wa)
                            else:
                                nc.scalar.activation(
                                    wa_sb[:, 2 * p + 1], ps2[:, 1], _IDENT,
                                    bias=0.0, scale=c_wa)
                    if not tail:
                        nc.scalar.activation(wa_sb[:, 2 * p:2 * p + 2], ps2,
                                             _IDENT, bias=0.0, scale=c_wa)

            def emit_out_ch(s, ch, tail=False):
                """out[o,n] = c_out * WaT^T @ g + xf for one chunk.

                Tiles alternate between two drain paths so neither engine
                nor psum slot-set serializes: (a) DVE STT straight from
                PSUM into the pd slots; (b) ACT scale (frees the pa slot)
                + Pool in-place xf add (Pool cannot read PSUM)."""
                d = st[s]
                wa_sb, g_sb, xf_sb = d["wa_sb"], d["g_sb"], d["xf_sb"]
                o_sb = d["o_sb"]
                for p in range(CB // 2):
                    act_pool = ((ch + p) % 2 == 1) if not tail else (p == 0)
                    ps2 = pa_tile() if act_pool else pd_tile()
                    for j in range(2):
                        ob = 2 * p + j
                        for k in range(CB // 2):
                            nc.tensor.matmul(
                                ps2[:, j, :CHW],
                                wa_sb[:, 2 * k:2 * k + 2,
                                      ob * P:(ob + 1) * P],
                                g_sb[:, ch, 2 * k:2 * k + 2, :],
                                start=(k == 0), stop=(k == CB // 2 - 1),
                                perf_mode=_DR)
                    osl = o_sb[:, ch, 2 * p:2 * p + 2]
                    xsl = xf_sb[:, ch, 2 * p:2 * p + 2]
                    if act_pool:
                        nc.scalar.activation(osl, ps2[:, :, :CHW], _IDENT,
                                             bias=0.0, scale=c_out)
                        nc.gpsimd.tensor_add(osl, osl, xsl)
                    else:
                        nc.vector.scalar_tensor_tensor(
                            osl, ps2[:, :, :CHW], c_out, xsl, _mult, _add)
                    if tail:
                        nc.sync.dma_start(out4[s][:, ch, 2 * p:2 * p + 2],
                                          osl)
                if not tail:
                    nc.sync.dma_start(out4[s][:, ch], o_sb[:, ch])

            def alloc_out(s):
                st[s]["o_sb"] = opool.tile([P, NCH, CB, CHW], _bf16,
                                           tag="osb", name="o_sb")

            # ---- schedule ----
            dma_head()
            emit_warmup(13, 5)
            emit_thpg_A(0)
            emit_theta(0, 3)
            emit_phi(0, 3)
            emit_theta(0, 4)
            emit_phi(0, 4)
            emit_g(0, 3, [0, 1])
            emit_g(0, 3, [2])
            for s in range(BL):
                if s + 2 < BL:
                    dma_sample(s + 2)
                last = (s == BL - 1)
                emit_att(s, tail=last)
                alloc_out(s)
                if not last:
                    emit_thpg_A(s + 1)
                    emit_theta(s + 1, 3)
                    emit_phi(s + 1, 3)
                    emit_wa(s)
                    emit_theta(s + 1, 4)
                    emit_phi(s + 1, 4)
                    emit_g(s + 1, 3, [0, 1])
                    emit_g(s + 1, 3, [2])
                    if s + 1 != BL - 1:
                        for ch in range(NCH):
                            emit_out_ch(s, ch)
                    # else: defer out(s) into the final iteration as filler
                else:
                    emit_out_ch(s - 1, 0)
                    emit_wa(s, tail=True)
                    emit_out_ch(s - 1, 1)
                    emit_out_ch(s - 1, 2)
                    for ch in range(NCH):
                        emit_out_ch(s, ch, tail=True)

    nc.finalize()
    return nc


def _get_program():
    global _PROGRAM
    if _PROGRAM is None:
        _PROGRAM = _build_program()
    return _PROGRAM


def _q8(a, scale):
    return np.asarray(a.astype(np.float32) * np.float32(scale)).astype(FP8NP)


def _prep_inputs(x, x_h, Wg, bg, Wt, bt, Wp, bp, Ww, bw, gamma, beta,
                 run_mean, run_var):
    f32 = np.float32
    inv = (gamma / np.sqrt(run_var + 1e-5)).astype(f32)
    off = ((bw - run_mean) * inv + beta).astype(f32)

    xr = np.ascontiguousarray(x.reshape(B, CB, P, N), dtype=f32)
    xhr = np.ascontiguousarray(x_h.reshape(B, CB, P, N), dtype=f32)

    wt_eff = np.ascontiguousarray(Wt.T).astype(f32) / f32(DIM)   # [C, O]
    wp_eff = np.ascontiguousarray(Wp.T).astype(f32)
    wg_eff = np.ascontiguousarray(Wg.T).astype(f32)
    ww_eff = np.ascontiguousarray(Ww.T * inv[None, :]).astype(f32)

    x0 = xr[0].reshape(DIM, N)
    xh0 = xhr[0].reshape(DIM, N)
    th0 = wt_eff.T @ x0 + (bt.astype(f32) / f32(DIM))[:, None]
    ph0 = wp_eff.T @ xh0 + bp.astype(f32)[:, None]
    g0 = wg_eff.T @ xh0 + bg.astype(f32)[:, None]
    at0 = th0 @ ph0.T                   # att[i, j]
    wa0 = at0.T @ ww_eff                # WaT[j, o]
    MARG = f32(1.45)

    def s_of(a, marg=MARG):
        return f32(FP8TGT / (np.abs(a).max() * marg))

    s_x = s_of(xr, f32(1.0))
    s_xh = s_of(xhr, f32(1.0))
    s_wt = s_of(wt_eff, f32(1.0))
    s_wp = s_of(wp_eff, f32(1.0))
    s_wg = s_of(wg_eff, f32(1.0))
    s_ww = s_of(ww_eff, f32(1.0))
    s_th = s_of(th0)
    s_ph = s_of(ph0)
    s_g = s_of(g0)
    s_at = s_of(at0)
    s_wa = s_of(wa0)

    wstack = np.stack([
        _q8(wt_eff.reshape(CB, P, DIM), s_wt),
        _q8(wp_eff.reshape(CB, P, DIM), s_wp),
        _q8(wg_eff.reshape(CB, P, DIM), s_wg),
        _q8(ww_eff.reshape(CB, P, DIM), s_ww),
    ])                                          # [4, CB, P, DIM]
    wall = np.ascontiguousarray(wstack.transpose(2, 0, 1, 3))  # [P, 4, CB, DIM]

    brow = np.zeros((1, 2, DIM), dtype=BF16NP)
    brow[0, 0, :] = (bt.astype(f32) * (s_th / f32(DIM))).astype(BF16NP)
    brow[0, 1, :] = (bp.astype(f32) * s_ph).astype(BF16NP)

    ccols = np.zeros((P, 16), dtype=f32)
    ccols[:, 0:4] = bg.astype(f32).reshape(CB, P).T * f32(s_g)
    ccols[:, 4] = s_th / (s_x * s_wt)      # c_theta
    ccols[:, 5] = s_ph / (s_xh * s_wp)     # c_phi
    ccols[:, 6] = s_at / (s_th * s_ph)     # c_att
    ccols[:, 7] = s_g / (s_xh * s_wg)      # c_g
    ccols[:, 8] = s_wa / (s_at * s_ww)     # c_wa
    ccols[:, 9] = f32(1.0) / (s_wa * s_g)  # c_out

    shared = dict(wall=wall, brow=brow, ccols=ccols)

    def pmajor(a):
        # [BL, CB, P, N] -> [BL, P, NCH, CB, CHW] (chunk-major)
        a = a.reshape(a.shape[0], CB, a.shape[2], NCH, CHW)
        return np.ascontiguousarray(a.transpose(0, 2, 3, 1, 4))

    in_maps = []
    for k in range(NCORES):
        m = dict(shared)
        sl = slice(k * BL, (k + 1) * BL)
        m["xf"] = pmajor(xr[sl] + off.reshape(1, CB, P, 1)).astype(BF16NP)
        xq = _q8(xr[sl], s_x).reshape(BL, CB, P, NCH, CHW)
        xhq = _q8(xhr[sl], s_xh).reshape(BL, CB, P, NCH, CHW)
        xi = np.stack([xq, xhq], axis=1)        # [BL, 2, CB, P, NCH, CHW]
        m["xin"] = np.ascontiguousarray(xi.transpose(0, 3, 4, 1, 2, 5))
        in_maps.append(m)
    return in_maps


def run(inputs, trace=False, tmpdir=None):
    nc = _get_program()
    in_maps = _prep_inputs(**inputs)
    res = bass_utils.run_bass_kernel_spmd(
        nc, in_maps, core_ids=list(range(NCORES)), trace=trace, tmpdir=tmpdir)
    outs = [np.asarray(r["out4"]).astype(np.float32) for r in res.results]
    out = np.concatenate(outs, axis=0).transpose(0, 3, 1, 2, 4)  # [B,CB,P,NCH,CHW]
    out = np.ascontiguousarray(out).reshape(B, DIM, H, W)
    return out, res


def kernel(**inputs) -> np.ndarray:
    out, _ = run(inputs)
    return out


# revision 21
# speedup vs baseline: 1.0607x; 1.0607x over previous
"""Trainium2 Bass kernel for the sparse_attention (channel-attention) module.

Computation per sample (x_s, xh_s are [512, 1152] slices):
    theta = Wt @ x_s  + bt        (fold 1/512 into Wt, bt)
    phi   = Wp @ xh_s + bp
    g     = Wg @ xh_s + bg
    att   = theta @ phi^T         (contract over n; includes the /512)
    Wa    = (Ww*inv) @ att        (BN scale folded into Ww; 512^3 GEMM,
                                   cheaper than y = att @ g at 512^2*1152)
    out   = Wa @ g + xf           (xf = x + BN offset, bf16, host-folded)

Sharding: pure data parallel, 4 samples per core across 8 cores.

All GEMMs run in fp8 (e4m3, max 240) with DoubleRow perf mode. Per-tensor
scales come from a host sample-0 forward estimate with margin, so the
compiled program is data-independent. theta/phi are produced transposed
(thetaT[n,i]); att is produced as att[i,j] (stationary thetaT) which is the
stationary operand Wa needs, and WaT[j,o] is the stationary operand of the
out matmuls.

Schedule notes (from trace analysis):
- DMA engines are shared round-robin across ACTIVE rings, so bulk prefetch
  must queue BEHIND head-critical transfers. Inputs ride the sync HW ring
  in need-order (then bulk xf/next-sample FIFO behind); weights + out
  writes ride the scalar HW ring. The slow-start gpsimd SW ring is unused.
- PSUM: tag "pd" (DVE-evicted: theta/phi/out) and "pa" (ACT: g/att/Wa) get
  2 two-bank slots each so slot reuse always waits on the matching engine,
  with >=1.7us reuse distance in the emission order below.
- out evictions alternate DVE / Pool(gpsimd) so the out phase is not
  DVE-serial-bound; Wa sits between theta/phi p3 and p4 of the next
  sample's thpg so its ACT evictions hide under DVE-evicted fills.
"""

import numpy as np
import ml_dtypes

import concourse.bass as bass
import concourse.mybir as mybir
from concourse import bacc
from concourse.tile import TileContext
from concourse import bass_utils

B, DIM, H, W = 32, 512, 48, 24
N = H * W            # 1152
P = 128
CB = DIM // P        # 4 channel blocks
NB = N // P          # 9 n blocks
NCH = 3
CHW = N // NCH       # 384
NCORES = 8
BL = B // NCORES     # 4 samples per core

_f32 = mybir.dt.float32
_bf16 = mybir.dt.bfloat16
_fp8 = mybir.dt.float8e4
_add = mybir.AluOpType.add
_mult = mybir.AluOpType.mult
_DR = mybir.MatmulPerfMode.DoubleRow
_IDENT = mybir.ActivationFunctionType.Identity

FP8NP = ml_dtypes.float8_e4m3
BF16NP = ml_dtypes.bfloat16
FP8TGT = 192.0                      # of 240 max: saturation headroom

_PROGRAM = None


def _build_program():
    nc = bacc.Bacc("TRN2", target_bir_lowering=False, debug=False)

    # x and x_h fp8, chunk-interleaved: xin[s][:, c, 0] = x chunk c,
    # xin[s][:, c, 1] = x_h chunk c.
    xin = nc.dram_tensor("xin", [BL, P, NCH, 2, CB, CHW], _fp8,
                         kind="ExternalInput").ap()
    xf = nc.dram_tensor("xf", [BL, P, NCH, CB, CHW], _bf16,
                        kind="ExternalInput").ap()
    wall = nc.dram_tensor("wall", [P, 4, CB, DIM], _fp8,
                          kind="ExternalInput").ap()
    # bias rows (pre-scaled to consumer fp8 grid): a single-partition 2KB
    # tensor, broadcast across partitions on-device via a 1-contraction
    # matmul (saves 0.5MB of head-critical DMA)
    brow = nc.dram_tensor("brow", [1, 2, DIM], _bf16,
                          kind="ExternalInput").ap()
    # per-partition columns: bg*s_g per o-block (0:4), eviction scales (4:10)
    ccols = nc.dram_tensor("ccols", [P, 16], _f32, kind="ExternalInput").ap()
    out4 = nc.dram_tensor("out4", [BL, P, NCH, CB, CHW], _bf16,
                          kind="ExternalOutput").ap()

    with TileContext(nc) as tc:
        with tc.tile_pool(name="const", bufs=1) as cpool, \
             tc.tile_pool(name="xin", bufs=3) as xpool, \
             tc.tile_pool(name="xfin", bufs=3) as xfpool, \
             tc.tile_pool(name="work", bufs=6) as wpool, \
             tc.tile_pool(name="attwa", bufs=4) as apool, \
             tc.tile_pool(name="out", bufs=2) as opool, \
             tc.tile_pool(name="psum", bufs=2, space="PSUM") as psum:

            ccols_sb = cpool.tile([P, 16], _f32, tag="ccols")
            cb2 = cpool.tile([P, 2, 2, DIM], _bf16, tag="cbias")
            row_sb = cpool.tile([P, 2, DIM], _bf16, tag="brow")
            ones_sb = cpool.tile([P, P], _bf16, tag="ones")
            w_sb = cpool.tile([P, 4, CB, DIM], _fp8, tag="wall")
            wt_sb = w_sb[:, 0]
            wp_sb = w_sb[:, 1]
            wg_sb = w_sb[:, 2]
            ww_sb = w_sb[:, 3]

            btb2 = cb2[:, 0]               # [P, 2, DIM]
            btb = cb2[:, 0, 0]             # [P, DIM]
            bpb2 = cb2[:, 1]
            bpb = cb2[:, 1, 0]
            bgc = [ccols_sb[:, i:i + 1] for i in range(0, 4)]
            c_theta = ccols_sb[:, 4:5]
            c_phi = ccols_sb[:, 5:6]
            c_att = ccols_sb[:, 6:7]
            c_g = ccols_sb[:, 7:8]
            c_wa = ccols_sb[:, 8:9]
            c_out = ccols_sb[:, 9:10]

            st = [dict() for _ in range(BL)]

            def pd_tile():
                return psum.tile([P, 2, DIM], _f32, tag="pd", name="pd")

            def pa_tile():
                return psum.tile([P, 2, DIM], _f32, tag="pa", name="pa")

            def emit_warmup(n, nbias):
                """Dummy matmuls on a zeroed tile: keeps the PE continuously
                busy from the preamble until real data lands, so the p-state
                ramp (0.65 -> 1.2 -> 2.4GHz after 3us continuous) completes
                before the first real matmul. After `nbias` dummies, the PE
                broadcasts the bias rows (partition 0 of row_sb) across all
                partitions into a psum tile; ACT copies them into the
                duplicated SBUF layout the paired theta/phi evictions use."""
                zt = cpool.tile([P, 2, CHW], _fp8, tag="warm")
                nc.vector.memset(zt, 0)
                nc.vector.memset(ones_sb, 1.0)
                wps = pa_tile()
                bps = pd_tile()
                for i in range(n):
                    nc.tensor.matmul(wps[:, i % 2, :CHW], zt[:, :, 0:P], zt,
                                     start=True, stop=True, perf_mode=_DR)
                    if i == nbias:
                        for j in range(2):
                            nc.tensor.matmul(bps[:, j], ones_sb[0:1],
                                             row_sb[0:1, j],
                                             start=True, stop=True)
                        nc.scalar.activation(cb2[:, :, 0], bps, _IDENT,
                                             bias=0.0, scale=1.0)
                        nc.scalar.activation(cb2[:, :, 1], bps, _IDENT,
                                             bias=0.0, scale=1.0)

            def dma_head():
                """Both HW rings, need-order; bulk FIFOs behind."""
                x_sb = xpool.tile([P, NCH, 2, CB, CHW], _fp8, tag="xin",
                                  name="x_sb")
                xf_sb = xfpool.tile([P, NCH, CB, CHW], _bf16, tag="xf",
                                    name="xf_sb")
                st[0].update(x_sb=x_sb, xf_sb=xf_sb)
                x1 = xpool.tile([P, NCH, 2, CB, CHW], _fp8, tag="xin",
                                name="x_sb")
                xf1 = xfpool.tile([P, NCH, CB, CHW], _bf16, tag="xf",
                                  name="xf_sb")
                st[1].update(x_sb=x1, xf_sb=xf1)
                # sync HW ring (fast): consts, then x/xh halves in use
                # order, then the bulk residuals FIFO behind
                nc.sync.dma_start(ccols_sb, ccols)
                nc.sync.dma_start(row_sb[0:1], brow)
                nc.sync.dma_start(x_sb[:, 0, 0], xin[0][:, 0, 0])
                nc.sync.dma_start(x_sb[:, 0, 1], xin[0][:, 0, 1])
                nc.sync.dma_start(x_sb[:, 1, 0], xin[0][:, 1, 0])
                nc.sync.dma_start(x_sb[:, 1, 1], xin[0][:, 1, 1])
                nc.sync.dma_start(x_sb[:, 2, 0], xin[0][:, 2, 0])
                nc.sync.dma_start(x_sb[:, 2, 1], xin[0][:, 2, 1])
                nc.sync.dma_start(xf_sb, xf[0])
                nc.sync.dma_start(x1, xin[1])
                nc.sync.dma_start(xf1, xf[1])
                # scalar ring (parallel): weights in two fat-line (4KB)
                # transfers
                nc.scalar.dma_start(w_sb[:, 0:2], wall[:, 0:2])
                nc.scalar.dma_start(w_sb[:, 2:4], wall[:, 2:4])

            def dma_sample(s):
                x_sb = xpool.tile([P, NCH, 2, CB, CHW], _fp8, tag="xin",
                                  name="x_sb")
                xf_sb = xfpool.tile([P, NCH, CB, CHW], _bf16, tag="xf",
                                    name="xf_sb")
                nc.sync.dma_start(x_sb, xin[s])
                nc.sync.dma_start(xf_sb, xf[s])
                st[s].update(x_sb=x_sb, xf_sb=xf_sb)

            def alloc_work(s):
                d = st[s]
                d["thetaT"] = wpool.tile([P, NB, DIM], _fp8, tag="work",
                                         name="thetaT")
                d["phiT"] = wpool.tile([P, NB, DIM], _fp8, tag="work",
                                       name="phiT")
                d["g_sb"] = wpool.tile([P, NCH, CB, CHW], _fp8, tag="work",
                                       name="g_sb")

            def conv_pair(x_sb, half, w, nbs):
                ps2 = pd_tile()
                for j, nb in enumerate(nbs):
                    c, jj = divmod(nb, NCH)
                    for k in range(CB // 2):
                        nc.tensor.matmul(
                            ps2[:, j],
                            x_sb[:, c, half, 2 * k:2 * k + 2,
                                 jj * P:(jj + 1) * P],
                            w[:, 2 * k:2 * k + 2],
                            start=(k == 0), stop=(k == CB // 2 - 1),
                            perf_mode=_DR)
                return ps2

            def emit_theta(s, p):
                d = st[s]
                nbs = [2 * p, 2 * p + 1] if 2 * p + 1 < NB else [NB - 1]
                ps2 = conv_pair(d["x_sb"], 0, wt_sb, nbs)
                if len(nbs) == 2:
                    nc.vector.scalar_tensor_tensor(
                        d["thetaT"][:, 2 * p:2 * p + 2], ps2, c_theta, btb2,
                        _mult, _add)
                else:
                    nc.vector.scalar_tensor_tensor(
                        d["thetaT"][:, NB - 1], ps2[:, 0], c_theta, btb,
                        _mult, _add)

            def emit_phi(s, p):
                d = st[s]
                nbs = [2 * p, 2 * p + 1] if 2 * p + 1 < NB else [NB - 1]
                ps2 = conv_pair(d["x_sb"], 1, wp_sb, nbs)
                if len(nbs) == 2:
                    nc.vector.scalar_tensor_tensor(
                        d["phiT"][:, 2 * p:2 * p + 2], ps2, c_phi, bpb2,
                        _mult, _add)
                else:
                    nc.vector.scalar_tensor_tensor(
                        d["phiT"][:, NB - 1], ps2[:, 0], c_phi, bpb,
                        _mult, _add)

            def emit_g(s, ob, chs):
                d = st[s]
                ps2 = pa_tile()
                for j, ch in enumerate(chs):
                    for k in range(CB // 2):
                        nc.tensor.matmul(
                            ps2[:, j, :CHW],
                            wg_sb[:, 2 * k:2 * k + 2, ob * P:(ob + 1) * P],
                            d["x_sb"][:, ch, 1, 2 * k:2 * k + 2, :],
                            start=(k == 0), stop=(k == CB // 2 - 1),
                            perf_mode=_DR)
                if len(chs) == 2:
                    nc.scalar.activation(
                        d["g_sb"][:, 0:2, ob], ps2[:, :, :CHW], _IDENT,
                        bias=bgc[ob], scale=c_g)
                else:
                    nc.scalar.activation(
                        d["g_sb"][:, chs[0], ob], ps2[:, 0, :CHW], _IDENT,
                        bias=bgc[ob], scale=c_g)

            def emit_thpg_A(s):
                alloc_work(s)
                for p in range(3):
                    emit_theta(s, p)
                    emit_phi(s, p)
                    emit_g(s, p, [0, 1])
                    emit_g(s, p, [2])

            def emit_att(s, tail=False):
                """att[i,j] = c_att * thetaT^T @ phiT (stationary thetaT)."""
                d = st[s]
                thetaT, phiT = d["thetaT"], d["phiT"]
                att_sb = apool.tile([P, CB, DIM], _fp8, tag="attwa",
                                    name="att_sb")
                d["att_sb"] = att_sb
                for p in range(CB // 2):
                    ps2 = pa_tile()
                    for j in range(2):
                        ib = 2 * p + j
                        for k in range(NB // 2):
                            nc.tensor.matmul(
                                ps2[:, j],
                                thetaT[:, 2 * k:2 * k + 2,
                                       ib * P:(ib + 1) * P],
                                phiT[:, 2 * k:2 * k + 2],
                                start=(k == 0), stop=False, perf_mode=_DR)
                        nc.tensor.matmul(
                            ps2[:, j], thetaT[:, NB - 1, ib * P:(ib + 1) * P],
                            phiT[:, NB - 1], start=False, stop=True)
                        if tail:
                            if j == 0:
                                nc.vector.tensor_scalar_mul(
                                    att_sb[:, 2 * p], ps2[:, 0], c_att)
                            else:
                                nc.scalar.activation(
                                    att_sb[:, 2 * p + 1], ps2[:, 1], _IDENT,
                                    bias=0.0, scale=c_att)
                    if not tail:
                        nc.scalar.activation(att_sb[:, 2 * p:2 * p + 2], ps2,
                                             _IDENT, bias=0.0, scale=c_att)

            def emit_wa(s, tail=False):
                """WaT[j,o] = c_wa * att^T @ (Ww*inv) (stationary att)."""
                d = st[s]
                att_sb = d["att_sb"]
                wa_sb = apool.tile([P, CB, DIM], _fp8, tag="attwa",
                                   name="wa_sb")
                d["wa_sb"] = wa_sb
                for p in range(CB // 2):
                    ps2 = pa_tile()
                    for j in range(2):
                        jb = 2 * p + j
                        for k in range(CB // 2):
                            nc.tensor.matmul(
                                ps2[:, j],
                                att_sb[:, 2 * k:2 * k + 2,
                                       jb * P:(jb + 1) * P],
                                ww_sb[:, 2 * k:2 * k + 2],
                                start=(k == 0), stop=(k == CB // 2 - 1),
                                perf_mode=_DR)
                        if tail:
                            if j == 0:
                                nc.vector.tensor_scalar_mul(
                                    wa_sb[:, 2 * p], ps2[:, 0], c_wa)
                            else:
                                nc.scalar.activation(
                                    wa_sb[:, 2 * p + 1], ps2[:, 1], _IDENT,
                                    bias=0.0, scale=c_wa)
                    if not tail:
                        nc.scalar.activation(wa_sb[:, 2 * p:2 * p + 2], ps2,
                                             _IDENT, bias=0.0, scale=c_wa)

            def emit_out_ch(s, ch, tail=False):
                """out[o,n] = c_out * WaT^T @ g + xf for one chunk.

                Tiles alternate between two drain paths so neither engine
                nor psum slot-set serializes: (a) DVE STT straight from
                PSUM into the pd slots; (b) ACT scale (frees the pa slot)
                + Pool in-place xf add (Pool cannot read PSUM)."""
                d = st[s]
                wa_sb, g_sb, xf_sb = d["wa_sb"], d["g_sb"], d["xf_sb"]
                o_sb = d["o_sb"]
                for p in range(CB // 2):
                    act_pool = ((ch + p) % 2 == 1) if not tail else (p == 0)
                    ps2 = pa_tile() if act_pool else pd_tile()
                    for j in range(2):
                        ob = 2 * p + j
                        for k in range(CB // 2):
                            nc.tensor.matmul(
                                ps2[:, j, :CHW],
                                wa_sb[:, 2 * k:2 * k + 2,
                                      ob * P:(ob + 1) * P],
                                g_sb[:, ch, 2 * k:2 * k + 2, :],
                                start=(k == 0), stop=(k == CB // 2 - 1),
                                perf_mode=_DR)
                    osl = o_sb[:, ch, 2 * p:2 * p + 2]
                    xsl = xf_sb[:, ch, 2 * p:2 * p + 2]
                    if act_pool:
                        nc.scalar.activation(osl, ps2[:, :, :CHW], _IDENT,
                                             bias=0.0, scale=c_out)
                        nc.gpsimd.tensor_add(osl, osl, xsl)
                    else:
                        nc.vector.scalar_tensor_tensor(
                            osl, ps2[:, :, :CHW], c_out, xsl, _mult, _add)
                    if tail:
                        nc.sync.dma_start(out4[s][:, ch, 2 * p:2 * p + 2],
                                          osl)
                if not tail:
                    nc.sync.dma_start(out4[s][:, ch], o_sb[:, ch])

            def alloc_out(s):
                st[s]["o_sb"] = opool.tile([P, NCH, CB, CHW], _bf16,
                                           tag="osb", name="o_sb")

            # ---- schedule ----
            dma_head()
            emit_warmup(13, 5)
            emit_thpg_A(0)
            emit_theta(0, 3)
            emit_phi(0, 3)
            emit_theta(0, 4)
            emit_phi(0, 4)
            emit_g(0, 3, [0, 1])
            emit_g(0, 3, [2])
            for s in range(BL):
                if s + 2 < BL:
                    dma_sample(s + 2)
                last = (s == BL - 1)
                emit_att(s, tail=last)
                alloc_out(s)
                if not last:
                    emit_thpg_A(s + 1)
                    emit_theta(s + 1, 3)
                    emit_phi(s + 1, 3)
                    emit_wa(s)
                    emit_theta(s + 1, 4)
                    emit_phi(s + 1, 4)
                    emit_g(s + 1, 3, [0, 1])
                    emit_g(s + 1, 3, [2])
                    if s + 1 != BL - 1:
                        for ch in range(NCH):
                            emit_out_ch(s, ch)
                    # else: defer out(s) into the final iteration as filler
                else:
                    emit_out_ch(s - 1, 0)
                    emit_wa(s, tail=True)
                    emit_out_ch(s - 1, 1)
                    emit_out_ch(s - 1, 2)
                    for ch in range(NCH):
                        emit_out_ch(s, ch, tail=True)

    nc.finalize()
    return nc


def _get_program():
    global _PROGRAM
    if _PROGRAM is None:
        _PROGRAM = _build_program()
    return _PROGRAM


def _q8(a, scale):
    return np.asarray(a.astype(np.float32) * np.float32(scale)).astype(FP8NP)


def _prep_inputs(x, x_h, Wg, bg, Wt, bt, Wp, bp, Ww, bw, gamma, beta,
                 run_mean, run_var):
    f32 = np.float32
    inv = (gamma / np.sqrt(run_var + 1e-5)).astype(f32)
    off = ((bw - run_mean) * inv + beta).astype(f32)

    xr = np.ascontiguousarray(x.reshape(B, CB, P, N), dtype=f32)
    xhr = np.ascontiguousarray(x_h.reshape(B, CB, P, N), dtype=f32)

    wt_eff = np.ascontiguousarray(Wt.T).astype(f32) / f32(DIM)   # [C, O]
    wp_eff = np.ascontiguousarray(Wp.T).astype(f32)
    wg_eff = np.ascontiguousarray(Wg.T).astype(f32)
    ww_eff = np.ascontiguousarray(Ww.T * inv[None, :]).astype(f32)

    x0 = xr[0].reshape(DIM, N)
    xh0 = xhr[0].reshape(DIM, N)
    th0 = wt_eff.T @ x0 + (bt.astype(f32) / f32(DIM))[:, None]
    ph0 = wp_eff.T @ xh0 + bp.astype(f32)[:, None]
    g0 = wg_eff.T @ xh0 + bg.astype(f32)[:, None]
    at0 = th0 @ ph0.T                   # att[i, j]
    wa0 = at0.T @ ww_eff                # WaT[j, o]
    MARG = f32(1.45)

    def s_of(a, marg=MARG):
        return f32(FP8TGT / (np.abs(a).max() * marg))

    s_x = s_of(xr, f32(1.0))
    s_xh = s_of(xhr, f32(1.0))
    s_wt = s_of(wt_eff, f32(1.0))
    s_wp = s_of(wp_eff, f32(1.0))
    s_wg = s_of(wg_eff, f32(1.0))
    s_ww = s_of(ww_eff, f32(1.0))
    s_th = s_of(th0)
    s_ph = s_of(ph0)
    s_g = s_of(g0)
    s_at = s_of(at0)
    s_wa = s_of(wa0)

    wstack = np.stack([
        _q8(wt_eff.reshape(CB, P, DIM), s_wt),
        _q8(wp_eff.reshape(CB, P, DIM), s_wp),
        _q8(wg_eff.reshape(CB, P, DIM), s_wg),
        _q8(ww_eff.reshape(CB, P, DIM), s_ww),
    ])                                          # [4, CB, P, DIM]
    wall = np.ascontiguousarray(wstack.transpose(2, 0, 1, 3))  # [P, 4, CB, DIM]

    brow = np.zeros((1, 2, DIM), dtype=BF16NP)
    brow[0, 0, :] = (bt.astype(f32) * (s_th / f32(DIM))).astype(BF16NP)
    brow[0, 1, :] = (bp.astype(f32) * s_ph).astype(BF16NP)

    ccols = np.zeros((P, 16), dtype=f32)
    ccols[:, 0:4] = bg.astype(f32).reshape(CB, P).T * f32(s_g)
    ccols[:, 4] = s_th / (s_x * s_wt)      # c_theta
    ccols[:, 5] = s_ph / (s_xh * s_wp)     # c_phi
    ccols[:, 6] = s_at / (s_th * s_ph)     # c_att
    ccols[:, 7] = s_g / (s_xh * s_wg)      # c_g
    ccols[:, 8] = s_wa / (s_at * s_ww)     # c_wa
    ccols[:, 9] = f32(1.0) / (s_wa * s_g)  # c_out

    shared = dict(wall=wall, brow=brow, ccols=ccols)

    def pmajor(a):
        # [BL, CB, P, N] -> [BL, P, NCH, CB, CHW] (chunk-major)
        a = a.reshape(a.shape[0], CB, a.shape[2], NCH, CHW)
        return np.ascontiguousarray(a.transpose(0, 2, 3, 1, 4))

    in_maps = []
    for k in range(NCORES):
        m = dict(shared)
        sl = slice(k * BL, (k + 1) * BL)
        m["xf"] = pmajor(xr[sl] + off.reshape(1, CB, P, 1)).astype(BF16NP)
        xq = _q8(xr[sl], s_x).reshape(BL, CB, P, NCH, CHW)
        xhq = _q8(xhr[sl], s_xh).reshape(BL, CB, P, NCH, CHW)
        xi = np.stack([xq, xhq], axis=1)        # [BL, 2, CB, P, NCH, CHW]
        m["xin"] = np.ascontiguousarray(xi.transpose(0, 3, 4, 1, 2, 5))
        in_maps.append(m)
    return in_maps


def run(inputs, trace=False, tmpdir=None):
    nc = _get_program()
    in_maps = _prep_inputs(**inputs)
    res = bass_utils.run_bass_kernel_spmd(
        nc, in_maps, core_ids=list(range(NCORES)), trace=trace, tmpdir=tmpdir)
    outs = [np.asarray(r["out4"]).astype(np.float32) for r in res.results]
    out = np.concatenate(outs, axis=0).transpose(0, 3, 1, 2, 4)  # [B,CB,P,NCH,CHW]
    out = np.ascontiguousarray(out).reshape(B, DIM, H, W)
    return out, res


def kernel(**inputs) -> np.ndarray:
    out, _ = run(inputs)
    return out


# revision 24
# speedup vs baseline: 1.0939x; 1.0313x over previous
"""Trainium2 Bass kernel for the sparse_attention (channel-attention) module.

Computation per sample (x_s, xh_s are [512, 1152] slices):
    theta = Wt @ x_s  + bt        (fold 1/512 into Wt, bt)
    phi   = Wp @ xh_s + bp
    g     = Wg @ xh_s + bg
    att   = theta @ phi^T         (contract over n; includes the /512)
    Wa    = (Ww*inv) @ att        (BN scale folded into Ww; 512^3 GEMM,
                                   cheaper than y = att @ g at 512^2*1152)
    out   = Wa @ g + xf           (xf = x + BN offset, bf16, host-folded)

Sharding: pure data parallel, 4 samples per core across 8 cores.

All GEMMs run in fp8 (e4m3, max 240) with DoubleRow perf mode. Per-tensor
scales come from a host sample-0 forward estimate with margin, so the
compiled program is data-independent. theta/phi are produced transposed
(thetaT[n,i]); att is produced as att[i,j] (stationary thetaT) which is the
stationary operand Wa needs, and WaT[j,o] is the stationary operand of the
out matmuls.

Schedule notes (from trace analysis):
- DMA engines are shared round-robin across ACTIVE rings, so bulk prefetch
  must queue BEHIND head-critical transfers. Inputs ride the sync HW ring
  in need-order (then bulk xf/next-sample FIFO behind); weights + out
  writes ride the scalar HW ring. The slow-start gpsimd SW ring is unused.
- PSUM: tag "pd" (DVE-evicted: theta/phi/out) and "pa" (ACT: g/att/Wa) get
  2 two-bank slots each so slot reuse always waits on the matching engine,
  with >=1.7us reuse distance in the emission order below.
- out evictions alternate DVE / Pool(gpsimd) so the out phase is not
  DVE-serial-bound; Wa sits between theta/phi p3 and p4 of the next
  sample's thpg so its ACT evictions hide under DVE-evicted fills.
"""

import numpy as np
import ml_dtypes

import concourse.bass as bass
import concourse.mybir as mybir
from concourse import bacc
from concourse.tile import TileContext
from concourse import bass_utils

B, DIM, H, W = 32, 512, 48, 24
N = H * W            # 1152
P = 128
CB = DIM // P        # 4 channel blocks
NB = N // P          # 9 n blocks
NCH = 3
CHW = N // NCH       # 384
NCORES = 8
BL = B // NCORES     # 4 samples per core

_f32 = mybir.dt.float32
_bf16 = mybir.dt.bfloat16
_fp8 = mybir.dt.float8e4
_add = mybir.AluOpType.add
_mult = mybir.AluOpType.mult
_DR = mybir.MatmulPerfMode.DoubleRow
_IDENT = mybir.ActivationFunctionType.Identity

FP8NP = ml_dtypes.float8_e4m3
BF16NP = ml_dtypes.bfloat16
FP8TGT = 192.0                      # of 240 max: saturation headroom

_PROGRAM = None


def _build_program():
    nc = bacc.Bacc("TRN2", target_bir_lowering=False, debug=False)

    # x and x_h fp8, chunk-interleaved: xin[s][:, c, 0] = x chunk c,
    # xin[s][:, c, 1] = x_h chunk c.
    xin = nc.dram_tensor("xin", [BL, P, NCH, 2, CB, CHW], _fp8,
                         kind="ExternalInput").ap()
    xf = nc.dram_tensor("xf", [BL, P, NCH, CB, CHW], _bf16,
                        kind="ExternalInput").ap()
    wall = nc.dram_tensor("wall", [P, 4, CB, DIM], _fp8,
                          kind="ExternalInput").ap()
    # bias rows (pre-scaled to consumer fp8 grid): a single-partition 2KB
    # tensor, broadcast across partitions on-device via a 1-contraction
    # matmul (saves 0.5MB of head-critical DMA)
    brow = nc.dram_tensor("brow", [1, 2, DIM], _bf16,
                          kind="ExternalInput").ap()
    # per-partition columns: bg*s_g per o-block (0:4), eviction scales (4:10)
    ccols = nc.dram_tensor("ccols", [P, 16], _f32, kind="ExternalInput").ap()
    out4 = nc.dram_tensor("out4", [BL, P, NCH, CB, CHW], _bf16,
                          kind="ExternalOutput").ap()

    with TileContext(nc) as tc:
        with tc.tile_pool(name="const", bufs=1) as cpool, \
             tc.tile_pool(name="xin", bufs=3) as xpool, \
             tc.tile_pool(name="xfin", bufs=3) as xfpool, \
             tc.tile_pool(name="work", bufs=6) as wpool, \
             tc.tile_pool(name="attwa", bufs=4) as apool, \
             tc.tile_pool(name="out", bufs=2) as opool, \
             tc.tile_pool(name="psum", bufs=2, space="PSUM") as psum:

            ccols_sb = cpool.tile([P, 16], _f32, tag="ccols")
            cb2 = cpool.tile([P, 2, 2, DIM], _bf16, tag="cbias")
            row_sb = cpool.tile([P, 2, DIM], _bf16, tag="brow")
            ones_sb = cpool.tile([P, P], _bf16, tag="ones")
            w_sb = cpool.tile([P, 4, CB, DIM], _fp8, tag="wall")
            wt_sb = w_sb[:, 0]
            wp_sb = w_sb[:, 1]
            wg_sb = w_sb[:, 2]
            ww_sb = w_sb[:, 3]

            btb2 = cb2[:, 0]               # [P, 2, DIM]
            btb = cb2[:, 0, 0]             # [P, DIM]
            bpb2 = cb2[:, 1]
            bpb = cb2[:, 1, 0]
            bgc = [ccols_sb[:, i:i + 1] for i in range(0, 4)]
            c_theta = ccols_sb[:, 4:5]
            c_phi = ccols_sb[:, 5:6]
            c_att = ccols_sb[:, 6:7]
            c_g = ccols_sb[:, 7:8]
            c_wa = ccols_sb[:, 8:9]
            c_out = ccols_sb[:, 9:10]

            st = [dict() for _ in range(BL)]

            def pd_tile():
                return psum.tile([P, 2, DIM], _f32, tag="pd", name="pd")

            def pa_tile():
                return psum.tile([P, 2, DIM], _f32, tag="pa", name="pa")

            def emit_warmup(n, nbias):
                """Dummy matmuls on a zeroed tile: keeps the PE continuously
                busy from the preamble until real data lands, so the p-state
                ramp (0.65 -> 1.2 -> 2.4GHz after 3us continuous) completes
                before the first real matmul. After `nbias` dummies, the PE
                broadcasts the bias rows (partition 0 of row_sb) across all
                partitions into a psum tile; ACT copies them into the
                duplicated SBUF layout the paired theta/phi evictions use."""
                zt = cpool.tile([P, 2, CHW], _fp8, tag="warm")
                nc.vector.memset(zt, 0)
                nc.vector.memset(ones_sb, 1.0)
                wps = pa_tile()
                bps = pd_tile()
                for i in range(n):
                    nc.tensor.matmul(wps[:, i % 2, :CHW], zt[:, :, 0:P], zt,
                                     start=True, stop=True, perf_mode=_DR)
                    if i == nbias:
                        for j in range(2):
                            nc.tensor.matmul(bps[:, j], ones_sb[0:1],
                                             row_sb[0:1, j],
                                             start=True, stop=True)
                        nc.scalar.activation(cb2[:, :, 0], bps, _IDENT,
                                             bias=0.0, scale=1.0)
                        nc.scalar.activation(cb2[:, :, 1], bps, _IDENT,
                                             bias=0.0, scale=1.0)

            def dma_head():
                """Both HW rings, need-order; bulk FIFOs behind."""
                x_sb = xpool.tile([P, NCH, 2, CB, CHW], _fp8, tag="xin",
                                  name="x_sb")
                xf_sb = xfpool.tile([P, NCH, CB, CHW], _bf16, tag="xf",
                                    name="xf_sb")
                st[0].update(x_sb=x_sb, xf_sb=xf_sb)
                x1 = xpool.tile([P, NCH, 2, CB, CHW], _fp8, tag="xin",
                                name="x_sb")
                xf1 = xfpool.tile([P, NCH, CB, CHW], _bf16, tag="xf",
                                  name="xf_sb")
                st[1].update(x_sb=x1, xf_sb=xf1)
                # sync HW ring (fast): consts + wt, then x/xh halves in use
                # order, then the bulk residuals FIFO behind
                nc.sync.dma_start(ccols_sb, ccols)
                nc.sync.dma_start(row_sb[0:1], brow)
                nc.sync.dma_start(w_sb[:, 0], wall[:, 0])
                nc.sync.dma_start(x_sb[:, 0, 0], xin[0][:, 0, 0])
                nc.sync.dma_start(x_sb[:, 0, 1], xin[0][:, 0, 1])
                nc.sync.dma_start(x_sb[:, 1, 0], xin[0][:, 1, 0])
                nc.sync.dma_start(x_sb[:, 1, 1], xin[0][:, 1, 1])
                nc.sync.dma_start(x_sb[:, 2, 0], xin[0][:, 2, 0])
                nc.sync.dma_start(x_sb[:, 2, 1], xin[0][:, 2, 1])
                nc.sync.dma_start(xf_sb, xf[0])
                nc.sync.dma_start(x1, xin[1])
                nc.sync.dma_start(xf1, xf[1])
                # scalar ring (parallel): wp+wg as one fat-line (4KB)
                # transfer, then ww (needed only ~15us in)
                nc.scalar.dma_start(w_sb[:, 1:3], wall[:, 1:3])
                nc.scalar.dma_start(w_sb[:, 3], wall[:, 3])

            def dma_sample(s):
                x_sb = xpool.tile([P, NCH, 2, CB, CHW], _fp8, tag="xin",
                                  name="x_sb")
                xf_sb = xfpool.tile([P, NCH, CB, CHW], _bf16, tag="xf",
                                    name="xf_sb")
                nc.sync.dma_start(x_sb, xin[s])
                nc.sync.dma_start(xf_sb, xf[s])
                st[s].update(x_sb=x_sb, xf_sb=xf_sb)

            def alloc_work(s):
                d = st[s]
                d["thetaT"] = wpool.tile([P, NB, DIM], _fp8, tag="work",
                                         name="thetaT")
                d["phiT"] = wpool.tile([P, NB, DIM], _fp8, tag="work",
                                       name="phiT")
                d["g_sb"] = wpool.tile([P, NCH, CB, CHW], _fp8, tag="work",
                                       name="g_sb")

            def conv_pair(x_sb, half, w, nbs):
                ps2 = pd_tile()
                for j, nb in enumerate(nbs):
                    c, jj = divmod(nb, NCH)
                    for k in range(CB // 2):
                        nc.tensor.matmul(
                            ps2[:, j],
                            x_sb[:, c, half, 2 * k:2 * k + 2,
                                 jj * P:(jj + 1) * P],
                            w[:, 2 * k:2 * k + 2],
                            start=(k == 0), stop=(k == CB // 2 - 1),
                            perf_mode=_DR)
                return ps2

            def emit_theta(s, p):
                d = st[s]
                nbs = [2 * p, 2 * p + 1] if 2 * p + 1 < NB else [NB - 1]
                ps2 = conv_pair(d["x_sb"], 0, wt_sb, nbs)
                if len(nbs) == 2:
                    nc.vector.scalar_tensor_tensor(
                        d["thetaT"][:, 2 * p:2 * p + 2], ps2, c_theta, btb2,
                        _mult, _add)
                else:
                    nc.vector.scalar_tensor_tensor(
                        d["thetaT"][:, NB - 1], ps2[:, 0], c_theta, btb,
                        _mult, _add)

            def emit_phi(s, p):
                d = st[s]
                nbs = [2 * p, 2 * p + 1] if 2 * p + 1 < NB else [NB - 1]
                ps2 = conv_pair(d["x_sb"], 1, wp_sb, nbs)
                if len(nbs) == 2:
                    nc.vector.scalar_tensor_tensor(
                        d["phiT"][:, 2 * p:2 * p + 2], ps2, c_phi, bpb2,
                        _mult, _add)
                else:
                    nc.vector.scalar_tensor_tensor(
                        d["phiT"][:, NB - 1], ps2[:, 0], c_phi, bpb,
                        _mult, _add)

            def emit_g(s, ob, chs):
                d = st[s]
                ps2 = pa_tile()
                for j, ch in enumerate(chs):
                    for k in range(CB // 2):
                        nc.tensor.matmul(
                            ps2[:, j, :CHW],
                            wg_sb[:, 2 * k:2 * k + 2, ob * P:(ob + 1) * P],
                            d["x_sb"][:, ch, 1, 2 * k:2 * k + 2, :],
                            start=(k == 0), stop=(k == CB // 2 - 1),
                            perf_mode=_DR)
                if len(chs) == 2:
                    nc.scalar.activation(
                        d["g_sb"][:, 0:2, ob], ps2[:, :, :CHW], _IDENT,
                        bias=bgc[ob], scale=c_g)
                else:
                    nc.scalar.activation(
                        d["g_sb"][:, chs[0], ob], ps2[:, 0, :CHW], _IDENT,
                        bias=bgc[ob], scale=c_g)

            def emit_thpg_A(s):
                alloc_work(s)
                for p in range(3):
                    emit_theta(s, p)
                    emit_phi(s, p)
                    emit_g(s, p, [0, 1])
                    emit_g(s, p, [2])

            def emit_att(s, tail=False):
                """att[i,j] = c_att * thetaT^T @ phiT (stationary thetaT)."""
                d = st[s]
                thetaT, phiT = d["thetaT"], d["phiT"]
                att_sb = apool.tile([P, CB, DIM], _fp8, tag="attwa",
                                    name="att_sb")
                d["att_sb"] = att_sb
                for p in range(CB // 2):
                    ps2 = pa_tile()
                    for j in range(2):
                        ib = 2 * p + j
                        for k in range(NB // 2):
                            nc.tensor.matmul(
                                ps2[:, j],
                                thetaT[:, 2 * k:2 * k + 2,
                                       ib * P:(ib + 1) * P],
                                phiT[:, 2 * k:2 * k + 2],
                                start=(k == 0), stop=False, perf_mode=_DR)
                        nc.tensor.matmul(
                            ps2[:, j], thetaT[:, NB - 1, ib * P:(ib + 1) * P],
                            phiT[:, NB - 1], start=False, stop=True)
                        if tail:
                            if j == 0:
                                nc.vector.tensor_scalar_mul(
                                    att_sb[:, 2 * p], ps2[:, 0], c_att)
                            else:
                                nc.scalar.activation(
                                    att_sb[:, 2 * p + 1], ps2[:, 1], _IDENT,
                                    bias=0.0, scale=c_att)
                    if not tail:
                        nc.scalar.activation(att_sb[:, 2 * p:2 * p + 2], ps2,
                                             _IDENT, bias=0.0, scale=c_att)

            def emit_wa(s, tail=False):
                """WaT[j,o] = c_wa * att^T @ (Ww*inv) (stationary att)."""
                d = st[s]
                att_sb = d["att_sb"]
                wa_sb = apool.tile([P, CB, DIM], _fp8, tag="attwa",
                                   name="wa_sb")
                d["wa_sb"] = wa_sb
                for p in range(CB // 2):
                    ps2 = pa_tile()
                    for j in range(2):
                        jb = 2 * p + j
                        for k in range(CB // 2):
                            nc.tensor.matmul(
                                ps2[:, j],
                                att_sb[:, 2 * k:2 * k + 2,
                                       jb * P:(jb + 1) * P],
                                ww_sb[:, 2 * k:2 * k + 2],
                                start=(k == 0), stop=(k == CB // 2 - 1),
                                perf_mode=_DR)
                        if tail:
                            if j == 0:
                                nc.vector.tensor_scalar_mul(
                                    wa_sb[:, 2 * p], ps2[:, 0], c_wa)
                            else:
                                nc.scalar.activation(
                                    wa_sb[:, 2 * p + 1], ps2[:, 1], _IDENT,
                                    bias=0.0, scale=c_wa)
                    if not tail:
                        nc.scalar.activation(wa_sb[:, 2 * p:2 * p + 2], ps2,
                                             _IDENT, bias=0.0, scale=c_wa)

            def emit_out_ch(s, ch, tail=False):
                """out[o,n] = c_out * WaT^T @ g + xf for one chunk.

                Tiles alternate between two drain paths so neither engine
                nor psum slot-set serializes: (a) DVE STT straight from
                PSUM into the pd slots; (b) ACT scale (frees the pa slot)
                + Pool in-place xf add (Pool cannot read PSUM)."""
                d = st[s]
                wa_sb, g_sb, xf_sb = d["wa_sb"], d["g_sb"], d["xf_sb"]
                o_sb = d["o_sb"]
                for p in range(CB // 2):
                    if tail:
                        # tail: Pool adds only on early tiles so the final
                        # evict->DMA chain is a single fast DVE STT
                        act_pool = (p == 0 and ch < 2)
                    elif s == BL - 2:
                        # deferred out (runs inside the final iteration):
                        # all-DVE, keeping Pool free for the tail tiles
                        act_pool = False
                    else:
                        act_pool = (ch + p) % 2 == 1
                    ps2 = pa_tile() if act_pool else pd_tile()
                    for j in range(2):
                        ob = 2 * p + j
                        for k in range(CB // 2):
                            nc.tensor.matmul(
                                ps2[:, j, :CHW],
                                wa_sb[:, 2 * k:2 * k + 2,
                                      ob * P:(ob + 1) * P],
                                g_sb[:, ch, 2 * k:2 * k + 2, :],
                                start=(k == 0), stop=(k == CB // 2 - 1),
                                perf_mode=_DR)
                    osl = o_sb[:, ch, 2 * p:2 * p + 2]
                    xsl = xf_sb[:, ch, 2 * p:2 * p + 2]
                    if act_pool:
                        nc.scalar.activation(osl, ps2[:, :, :CHW], _IDENT,
                                             bias=0.0, scale=c_out)
                        nc.gpsimd.tensor_add(osl, osl, xsl)
                    else:
                        nc.vector.scalar_tensor_tensor(
                            osl, ps2[:, :, :CHW], c_out, xsl, _mult, _add)
                    if tail:
                        nc.sync.dma_start(out4[s][:, ch, 2 * p:2 * p + 2],
                                          osl)
                if not tail:
                    nc.sync.dma_start(out4[s][:, ch], o_sb[:, ch])

            def alloc_out(s):
                st[s]["o_sb"] = opool.tile([P, NCH, CB, CHW], _bf16,
                                           tag="osb", name="o_sb")

            # ---- schedule ----
            dma_head()
            emit_warmup(10, 4)
            emit_thpg_A(0)
            emit_theta(0, 3)
            emit_phi(0, 3)
            emit_theta(0, 4)
            emit_phi(0, 4)
            emit_g(0, 3, [0, 1])
            emit_g(0, 3, [2])
            for s in range(BL):
                if s + 2 < BL:
                    dma_sample(s + 2)
                last = (s == BL - 1)
                emit_att(s, tail=last)
                alloc_out(s)
                if not last:
                    emit_thpg_A(s + 1)
                    emit_theta(s + 1, 3)
                    emit_phi(s + 1, 3)
                    emit_wa(s)
                    emit_theta(s + 1, 4)
                    emit_phi(s + 1, 4)
                    emit_g(s + 1, 3, [0, 1])
                    emit_g(s + 1, 3, [2])
                    if s + 1 != BL - 1:
                        for ch in range(NCH):
                            emit_out_ch(s, ch)
                    # else: defer out(s) into the final iteration as filler
                else:
                    emit_out_ch(s - 1, 0)
                    emit_wa(s, tail=True)
                    emit_out_ch(s - 1, 1)
                    emit_out_ch(s - 1, 2)
                    for ch in range(NCH):
                        emit_out_ch(s, ch, tail=True)

    nc.finalize()
    return nc


def _get_program():
    global _PROGRAM
    if _PROGRAM is None:
        _PROGRAM = _build_program()
    return _PROGRAM


def _q8(a, scale):
    return np.asarray(a.astype(np.float32) * np.float32(scale)).astype(FP8NP)


def _prep_inputs(x, x_h, Wg, bg, Wt, bt, Wp, bp, Ww, bw, gamma, beta,
                 run_mean, run_var):
    f32 = np.float32
    inv = (gamma / np.sqrt(run_var + 1e-5)).astype(f32)
    off = ((bw - run_mean) * inv + beta).astype(f32)

    xr = np.ascontiguousarray(x.reshape(B, CB, P, N), dtype=f32)
    xhr = np.ascontiguousarray(x_h.reshape(B, CB, P, N), dtype=f32)

    wt_eff = np.ascontiguousarray(Wt.T).astype(f32) / f32(DIM)   # [C, O]
    wp_eff = np.ascontiguousarray(Wp.T).astype(f32)
    wg_eff = np.ascontiguousarray(Wg.T).astype(f32)
    ww_eff = np.ascontiguousarray(Ww.T * inv[None, :]).astype(f32)

    x0 = xr[0].reshape(DIM, N)
    xh0 = xhr[0].reshape(DIM, N)
    th0 = wt_eff.T @ x0 + (bt.astype(f32) / f32(DIM))[:, None]
    ph0 = wp_eff.T @ xh0 + bp.astype(f32)[:, None]
    g0 = wg_eff.T @ xh0 + bg.astype(f32)[:, None]
    at0 = th0 @ ph0.T                   # att[i, j]
    wa0 = at0.T @ ww_eff                # WaT[j, o]
    MARG = f32(1.45)

    def s_of(a, marg=MARG):
        return f32(FP8TGT / (np.abs(a).max() * marg))

    s_x = s_of(xr, f32(1.0))
    s_xh = s_of(xhr, f32(1.0))
    s_wt = s_of(wt_eff, f32(1.0))
    s_wp = s_of(wp_eff, f32(1.0))
    s_wg = s_of(wg_eff, f32(1.0))
    s_ww = s_of(ww_eff, f32(1.0))
    s_th = s_of(th0)
    s_ph = s_of(ph0)
    s_g = s_of(g0)
    s_at = s_of(at0)
    s_wa = s_of(wa0)

    wstack = np.stack([
        _q8(wt_eff.reshape(CB, P, DIM), s_wt),
        _q8(wp_eff.reshape(CB, P, DIM), s_wp),
        _q8(wg_eff.reshape(CB, P, DIM), s_wg),
        _q8(ww_eff.reshape(CB, P, DIM), s_ww),
    ])                                          # [4, CB, P, DIM]
    wall = np.ascontiguousarray(wstack.transpose(2, 0, 1, 3))  # [P, 4, CB, DIM]

    brow = np.zeros((1, 2, DIM), dtype=BF16NP)
    brow[0, 0, :] = (bt.astype(f32) * (s_th / f32(DIM))).astype(BF16NP)
    brow[0, 1, :] = (bp.astype(f32) * s_ph).astype(BF16NP)

    ccols = np.zeros((P, 16), dtype=f32)
    ccols[:, 0:4] = bg.astype(f32).reshape(CB, P).T * f32(s_g)
    ccols[:, 4] = s_th / (s_x * s_wt)      # c_theta
    ccols[:, 5] = s_ph / (s_xh * s_wp)     # c_phi
    ccols[:, 6] = s_at / (s_th * s_ph)     # c_att
    ccols[:, 7] = s_g / (s_xh * s_wg)      # c_g
    ccols[:, 8] = s_wa / (s_at * s_ww)     # c_wa
    ccols[:, 9] = f32(1.0) / (s_wa * s_g)  # c_out

    shared = dict(wall=wall, brow=brow, ccols=ccols)

    def pmajor(a):
        # [BL, CB, P, N] -> [BL, P, NCH, CB, CHW] (chunk-major)
        a = a.reshape(a.shape[0], CB, a.shape[2], NCH, CHW)
        return np.ascontiguousarray(a.transpose(0, 2, 3, 1, 4))

    in_maps = []
    for k in range(NCORES):
        m = dict(shared)
        sl = slice(k * BL, (k + 1) * BL)
        m["xf"] = pmajor(xr[sl] + off.reshape(1, CB, P, 1)).astype(BF16NP)
        xq = _q8(xr[sl], s_x).reshape(BL, CB, P, NCH, CHW)
        xhq = _q8(xhr[sl], s_xh).reshape(BL, CB, P, NCH, CHW)
        xi = np.stack([xq, xhq], axis=1)        # [BL, 2, CB, P, NCH, CHW]
        m["xin"] = np.ascontiguousarray(xi.transpose(0, 3, 4, 1, 2, 5))
        in_maps.append(m)
    return in_maps


def run(inputs, trace=False, tmpdir=None):
    nc = _get_program()
    in_maps = _prep_inputs(**inputs)
    res = bass_utils.run_bass_kernel_spmd(
        nc, in_maps, core_ids=list(range(NCORES)), trace=trace, tmpdir=tmpdir)
    outs = [np.asarray(r["out4"]).astype(np.float32) for r in res.results]
    out = np.concatenate(outs, axis=0).transpose(0, 3, 1, 2, 4)  # [B,CB,P,NCH,CHW]
    out = np.ascontiguousarray(out).reshape(B, DIM, H, W)
    return out, res


def kernel(**inputs) -> np.ndarray:
    out, _ = run(inputs)
    return out


# revision 25
# speedup vs baseline: 1.0965x; 1.0024x over previous
"""Trainium2 Bass kernel for the sparse_attention (channel-attention) module.

Computation per sample (x_s, xh_s are [512, 1152] slices):
    theta = Wt @ x_s  + bt        (fold 1/512 into Wt, bt)
    phi   = Wp @ xh_s + bp
    g     = Wg @ xh_s + bg
    att   = theta @ phi^T         (contract over n; includes the /512)
    Wa    = (Ww*inv) @ att        (BN scale folded into Ww; 512^3 GEMM,
                                   cheaper than y = att @ g at 512^2*1152)
    out   = Wa @ g + xf           (xf = x + BN offset, bf16, host-folded)

Sharding: pure data parallel, 4 samples per core across 8 cores.

All GEMMs run in fp8 (e4m3, max 240) with DoubleRow perf mode. Per-tensor
scales come from a host sample-0 forward estimate with margin, so the
compiled program is data-independent. theta/phi are produced transposed
(thetaT[n,i]); att is produced as att[i,j] (stationary thetaT) which is the
stationary operand Wa needs, and WaT[j,o] is the stationary operand of the
out matmuls.

Schedule notes (from trace analysis):
- DMA engines are shared round-robin across ACTIVE rings, so bulk prefetch
  must queue BEHIND head-critical transfers. Inputs ride the sync HW ring
  in need-order (then bulk xf/next-sample FIFO behind); weights + out
  writes ride the scalar HW ring. The slow-start gpsimd SW ring is unused.
- PSUM: tag "pd" (DVE-evicted: theta/phi/out) and "pa" (ACT: g/att/Wa) get
  2 two-bank slots each so slot reuse always waits on the matching engine,
  with >=1.7us reuse distance in the emission order below.
- out evictions alternate DVE / Pool(gpsimd) so the out phase is not
  DVE-serial-bound; Wa sits between theta/phi p3 and p4 of the next
  sample's thpg so its ACT evictions hide under DVE-evicted fills.
"""

import numpy as np
import ml_dtypes

import concourse.bass as bass
import concourse.mybir as mybir
from concourse import bacc
from concourse.tile import TileContext
from concourse import bass_utils

B, DIM, H, W = 32, 512, 48, 24
N = H * W            # 1152
P = 128
CB = DIM // P        # 4 channel blocks
NB = N // P          # 9 n blocks
NCH = 3
CHW = N // NCH       # 384
NCORES = 8
BL = B // NCORES     # 4 samples per core

_f32 = mybir.dt.float32
_bf16 = mybir.dt.bfloat16
_fp8 = mybir.dt.float8e4
_add = mybir.AluOpType.add
_mult = mybir.AluOpType.mult
_DR = mybir.MatmulPerfMode.DoubleRow
_IDENT = mybir.ActivationFunctionType.Identity

FP8NP = ml_dtypes.float8_e4m3
BF16NP = ml_dtypes.bfloat16
FP8TGT = 192.0                      # of 240 max: saturation headroom

_PROGRAM = None


def _build_program():
    nc = bacc.Bacc("TRN2", target_bir_lowering=False, debug=False)

    # x and x_h fp8, chunk-interleaved: xin[s][:, c, 0] = x chunk c,
    # xin[s][:, c, 1] = x_h chunk c.
    xin = nc.dram_tensor("xin", [BL, P, NCH, 2, CB, CHW], _fp8,
                         kind="ExternalInput").ap()
    xf = nc.dram_tensor("xf", [BL, P, NCH, CB, CHW], _bf16,
                        kind="ExternalInput").ap()
    wall = nc.dram_tensor("wall", [P, 4, CB, DIM], _fp8,
                          kind="ExternalInput").ap()
    # bias rows (pre-scaled to consumer fp8 grid): a single-partition 2KB
    # tensor, broadcast across partitions on-device via a 1-contraction
    # matmul (saves 0.5MB of head-critical DMA)
    brow = nc.dram_tensor("brow", [1, 2, DIM], _bf16,
                          kind="ExternalInput").ap()
    # per-partition columns: bg*s_g per o-block (0:4), eviction scales (4:10)
    ccols = nc.dram_tensor("ccols", [P, 16], _f32, kind="ExternalInput").ap()
    out4 = nc.dram_tensor("out4", [BL, P, NCH, CB, CHW], _bf16,
                          kind="ExternalOutput").ap()

    with TileContext(nc) as tc:
        with tc.tile_pool(name="const", bufs=1) as cpool, \
             tc.tile_pool(name="xin", bufs=3) as xpool, \
             tc.tile_pool(name="xfin", bufs=3) as xfpool, \
             tc.tile_pool(name="work", bufs=6) as wpool, \
             tc.tile_pool(name="attwa", bufs=4) as apool, \
             tc.tile_pool(name="out", bufs=2) as opool, \
             tc.tile_pool(name="psum", bufs=2, space="PSUM") as psum:

            ccols_sb = cpool.tile([P, 16], _f32, tag="ccols")
            cb2 = cpool.tile([P, 2, 2, DIM], _bf16, tag="cbias")
            row_sb = cpool.tile([P, 2, DIM], _bf16, tag="brow")
            ones_sb = cpool.tile([P, P], _bf16, tag="ones")
            w_sb = cpool.tile([P, 4, CB, DIM], _fp8, tag="wall")
            wt_sb = w_sb[:, 0]
            wp_sb = w_sb[:, 1]
            wg_sb = w_sb[:, 2]
            ww_sb = w_sb[:, 3]

            btb2 = cb2[:, 0]               # [P, 2, DIM]
            btb = cb2[:, 0, 0]             # [P, DIM]
            bpb2 = cb2[:, 1]
            bpb = cb2[:, 1, 0]
            bgc = [ccols_sb[:, i:i + 1] for i in range(0, 4)]
            c_theta = ccols_sb[:, 4:5]
            c_phi = ccols_sb[:, 5:6]
            c_att = ccols_sb[:, 6:7]
            c_g = ccols_sb[:, 7:8]
            c_wa = ccols_sb[:, 8:9]
            c_out = ccols_sb[:, 9:10]

            st = [dict() for _ in range(BL)]

            def pd_tile():
                return psum.tile([P, 2, DIM], _f32, tag="pd", name="pd")

            def pa_tile():
                return psum.tile([P, 2, DIM], _f32, tag="pa", name="pa")

            def emit_warmup(n, nbias):
                """Dummy matmuls on a zeroed tile: keeps the PE continuously
                busy from the preamble until real data lands, so the p-state
                ramp (0.65 -> 1.2 -> 2.4GHz after 3us continuous) completes
                before the first real matmul. After `nbias` dummies, the PE
                broadcasts the bias rows (partition 0 of row_sb) across all
                partitions into a psum tile; ACT copies them into the
                duplicated SBUF layout the paired theta/phi evictions use."""
                zt = cpool.tile([P, 2, CHW], _fp8, tag="warm")
                nc.vector.memset(zt, 0)
                nc.vector.memset(ones_sb, 1.0)
                wps = pa_tile()
                bps = pd_tile()
                for i in range(n):
                    nc.tensor.matmul(wps[:, i % 2, :CHW], zt[:, :, 0:P], zt,
                                     start=True, stop=True, perf_mode=_DR)
                    if i == nbias:
                        for j in range(2):
                            nc.tensor.matmul(bps[:, j], ones_sb[0:1],
                                             row_sb[0:1, j],
                                             start=True, stop=True)
                        nc.scalar.activation(cb2[:, :, 0], bps, _IDENT,
                                             bias=0.0, scale=1.0)
                        nc.scalar.activation(cb2[:, :, 1], bps, _IDENT,
                                             bias=0.0, scale=1.0)

            def dma_head():
                """Both HW rings, need-order; bulk FIFOs behind."""
                x_sb = xpool.tile([P, NCH, 2, CB, CHW], _fp8, tag="xin",
                                  name="x_sb")
                xf_sb = xfpool.tile([P, NCH, CB, CHW], _bf16, tag="xf",
                                    name="xf_sb")
                st[0].update(x_sb=x_sb, xf_sb=xf_sb)
                x1 = xpool.tile([P, NCH, 2, CB, CHW], _fp8, tag="xin",
                                name="x_sb")
                xf1 = xfpool.tile([P, NCH, CB, CHW], _bf16, tag="xf",
                                  name="xf_sb")
                st[1].update(x_sb=x1, xf_sb=xf1)
                # sync HW ring (fast): consts + wt, then x/xh halves in use
                # order, then the bulk residuals FIFO behind
                nc.sync.dma_start(ccols_sb, ccols)
                nc.sync.dma_start(row_sb[0:1], brow)
                nc.sync.dma_start(w_sb[:, 0], wall[:, 0])
                nc.sync.dma_start(x_sb[:, 0, 0], xin[0][:, 0, 0])
                nc.sync.dma_start(x_sb[:, 0, 1], xin[0][:, 0, 1])
                nc.sync.dma_start(x_sb[:, 1, 0], xin[0][:, 1, 0])
                nc.sync.dma_start(x_sb[:, 1, 1], xin[0][:, 1, 1])
                nc.sync.dma_start(x_sb[:, 2, 0], xin[0][:, 2, 0])
                nc.sync.dma_start(x_sb[:, 2, 1], xin[0][:, 2, 1])
                nc.sync.dma_start(xf_sb, xf[0])
                nc.sync.dma_start(x1, xin[1])
                nc.sync.dma_start(xf1, xf[1])
                # scalar ring (parallel): wp+wg as one fat-line (4KB)
                # transfer, then ww (needed only ~15us in)
                nc.scalar.dma_start(w_sb[:, 1:3], wall[:, 1:3])
                nc.scalar.dma_start(w_sb[:, 3], wall[:, 3])

            def dma_sample(s):
                x_sb = xpool.tile([P, NCH, 2, CB, CHW], _fp8, tag="xin",
                                  name="x_sb")
                xf_sb = xfpool.tile([P, NCH, CB, CHW], _bf16, tag="xf",
                                    name="xf_sb")
                nc.sync.dma_start(x_sb, xin[s])
                nc.sync.dma_start(xf_sb, xf[s])
                st[s].update(x_sb=x_sb, xf_sb=xf_sb)

            def alloc_work(s):
                d = st[s]
                d["thetaT"] = wpool.tile([P, NB, DIM], _fp8, tag="work",
                                         name="thetaT")
                d["phiT"] = wpool.tile([P, NB, DIM], _fp8, tag="work",
                                       name="phiT")
                d["g_sb"] = wpool.tile([P, NCH, CB, CHW], _fp8, tag="work",
                                       name="g_sb")

            def conv_pair(x_sb, half, w, nbs):
                ps2 = pd_tile()
                for j, nb in enumerate(nbs):
                    c, jj = divmod(nb, NCH)
                    for k in range(CB // 2):
                        nc.tensor.matmul(
                            ps2[:, j],
                            x_sb[:, c, half, 2 * k:2 * k + 2,
                                 jj * P:(jj + 1) * P],
                            w[:, 2 * k:2 * k + 2],
                            start=(k == 0), stop=(k == CB // 2 - 1),
                            perf_mode=_DR)
                return ps2

            def emit_theta(s, p):
                d = st[s]
                nbs = [2 * p, 2 * p + 1] if 2 * p + 1 < NB else [NB - 1]
                ps2 = conv_pair(d["x_sb"], 0, wt_sb, nbs)
                if len(nbs) == 2:
                    nc.vector.scalar_tensor_tensor(
                        d["thetaT"][:, 2 * p:2 * p + 2], ps2, c_theta, btb2,
                        _mult, _add)
                else:
                    nc.vector.scalar_tensor_tensor(
                        d["thetaT"][:, NB - 1], ps2[:, 0], c_theta, btb,
                        _mult, _add)

            def emit_phi(s, p):
                d = st[s]
                nbs = [2 * p, 2 * p + 1] if 2 * p + 1 < NB else [NB - 1]
                ps2 = conv_pair(d["x_sb"], 1, wp_sb, nbs)
                if len(nbs) == 2:
                    nc.vector.scalar_tensor_tensor(
                        d["phiT"][:, 2 * p:2 * p + 2], ps2, c_phi, bpb2,
                        _mult, _add)
                else:
                    nc.vector.scalar_tensor_tensor(
                        d["phiT"][:, NB - 1], ps2[:, 0], c_phi, bpb,
                        _mult, _add)

            def emit_g(s, ob, chs):
                d = st[s]
                ps2 = pa_tile()
                for j, ch in enumerate(chs):
                    for k in range(CB // 2):
                        nc.tensor.matmul(
                            ps2[:, j, :CHW],
                            wg_sb[:, 2 * k:2 * k + 2, ob * P:(ob + 1) * P],
                            d["x_sb"][:, ch, 1, 2 * k:2 * k + 2, :],
                            start=(k == 0), stop=(k == CB // 2 - 1),
                            perf_mode=_DR)
                if len(chs) == 2:
                    nc.scalar.activation(
                        d["g_sb"][:, 0:2, ob], ps2[:, :, :CHW], _IDENT,
                        bias=bgc[ob], scale=c_g)
                else:
                    nc.scalar.activation(
                        d["g_sb"][:, chs[0], ob], ps2[:, 0, :CHW], _IDENT,
                        bias=bgc[ob], scale=c_g)

            def emit_thpg_A(s):
                alloc_work(s)
                for p in range(3):
                    emit_theta(s, p)
                    emit_phi(s, p)
                    emit_g(s, p, [0, 1])
                    emit_g(s, p, [2])

            def emit_att(s, tail=False):
                """att[i,j] = c_att * thetaT^T @ phiT (stationary thetaT)."""
                d = st[s]
                thetaT, phiT = d["thetaT"], d["phiT"]
                att_sb = apool.tile([P, CB, DIM], _fp8, tag="attwa",
                                    name="att_sb")
                d["att_sb"] = att_sb
                for p in range(CB // 2):
                    ps2 = pa_tile()
                    for j in range(2):
                        ib = 2 * p + j
                        for k in range(NB // 2):
                            nc.tensor.matmul(
                                ps2[:, j],
                                thetaT[:, 2 * k:2 * k + 2,
                                       ib * P:(ib + 1) * P],
                                phiT[:, 2 * k:2 * k + 2],
                                start=(k == 0), stop=False, perf_mode=_DR)
                        nc.tensor.matmul(
                            ps2[:, j], thetaT[:, NB - 1, ib * P:(ib + 1) * P],
                            phiT[:, NB - 1], start=False, stop=True)
                        if tail:
                            if j == 0:
                                nc.vector.tensor_scalar_mul(
                                    att_sb[:, 2 * p], ps2[:, 0], c_att)
                            else:
                                nc.scalar.activation(
                                    att_sb[:, 2 * p + 1], ps2[:, 1], _IDENT,
                                    bias=0.0, scale=c_att)
                    if not tail:
                        nc.scalar.activation(att_sb[:, 2 * p:2 * p + 2], ps2,
                                             _IDENT, bias=0.0, scale=c_att)

            def emit_wa(s, tail=False):
                """WaT[j,o] = c_wa * att^T @ (Ww*inv) (stationary att)."""
                d = st[s]
                att_sb = d["att_sb"]
                wa_sb = apool.tile([P, CB, DIM], _fp8, tag="attwa",
                                   name="wa_sb")
                d["wa_sb"] = wa_sb
                for p in range(CB // 2):
                    ps2 = pa_tile()
                    for j in range(2):
                        jb = 2 * p + j
                        for k in range(CB // 2):
                            nc.tensor.matmul(
                                ps2[:, j],
                                att_sb[:, 2 * k:2 * k + 2,
                                       jb * P:(jb + 1) * P],
                                ww_sb[:, 2 * k:2 * k + 2],
                                start=(k == 0), stop=(k == CB // 2 - 1),
                                perf_mode=_DR)
                        if tail:
                            if j == 0:
                                nc.vector.tensor_scalar_mul(
                                    wa_sb[:, 2 * p], ps2[:, 0], c_wa)
                            else:
                                nc.scalar.activation(
                                    wa_sb[:, 2 * p + 1], ps2[:, 1], _IDENT,
                                    bias=0.0, scale=c_wa)
                    if not tail:
                        nc.scalar.activation(wa_sb[:, 2 * p:2 * p + 2], ps2,
                                             _IDENT, bias=0.0, scale=c_wa)

            def emit_out_ch(s, ch, tail=False):
                """out[o,n] = c_out * WaT^T @ g + xf for one chunk.

                Tiles alternate between two drain paths so neither engine
                nor psum slot-set serializes: (a) DVE STT straight from
                PSUM into the pd slots; (b) ACT scale (frees the pa slot)
                + Pool in-place xf add (Pool cannot read PSUM)."""
                d = st[s]
                wa_sb, g_sb, xf_sb = d["wa_sb"], d["g_sb"], d["xf_sb"]
                o_sb = d["o_sb"]
                for p in range(CB // 2):
                    if tail:
                        # tail: Pool adds only on early tiles so the final
                        # evict->DMA chain is a single fast DVE STT
                        act_pool = (p == 0 and ch < 2)
                    elif s == BL - 2:
                        # deferred out (runs inside the final iteration):
                        # all-DVE, keeping Pool free for the tail tiles
                        act_pool = False
                    else:
                        act_pool = (ch + p) % 2 == 1
                    ps2 = pa_tile() if act_pool else pd_tile()
                    for j in range(2):
                        ob = 2 * p + j
                        for k in range(CB // 2):
                            nc.tensor.matmul(
                                ps2[:, j, :CHW],
                                wa_sb[:, 2 * k:2 * k + 2,
                                      ob * P:(ob + 1) * P],
                                g_sb[:, ch, 2 * k:2 * k + 2, :],
                                start=(k == 0), stop=(k == CB // 2 - 1),
                                perf_mode=_DR)
                    osl = o_sb[:, ch, 2 * p:2 * p + 2]
                    xsl = xf_sb[:, ch, 2 * p:2 * p + 2]
                    if act_pool:
                        nc.scalar.activation(osl, ps2[:, :, :CHW], _IDENT,
                                             bias=0.0, scale=c_out)
                        nc.gpsimd.tensor_add(osl, osl, xsl)
                    else:
                        nc.vector.scalar_tensor_tensor(
                            osl, ps2[:, :, :CHW], c_out, xsl, _mult, _add)
                    if tail:
                        q = nc.sync if (2 * ch + p) % 2 == 0 else nc.scalar
                        q.dma_start(out4[s][:, ch, 2 * p:2 * p + 2], osl)
                if not tail:
                    nc.sync.dma_start(out4[s][:, ch], o_sb[:, ch])

            def alloc_out(s):
                st[s]["o_sb"] = opool.tile([P, NCH, CB, CHW], _bf16,
                                           tag="osb", name="o_sb")

            # ---- schedule ----
            dma_head()
            emit_warmup(10, 4)
            emit_thpg_A(0)
            emit_theta(0, 3)
            emit_phi(0, 3)
            emit_theta(0, 4)
            emit_phi(0, 4)
            emit_g(0, 3, [0, 1])
            emit_g(0, 3, [2])
            for s in range(BL):
                if s + 2 < BL:
                    dma_sample(s + 2)
                last = (s == BL - 1)
                emit_att(s, tail=last)
                alloc_out(s)
                if not last:
                    emit_thpg_A(s + 1)
                    emit_theta(s + 1, 3)
                    emit_phi(s + 1, 3)
                    emit_wa(s)
                    emit_theta(s + 1, 4)
                    emit_phi(s + 1, 4)
                    emit_g(s + 1, 3, [0, 1])
                    emit_g(s + 1, 3, [2])
                    if s + 1 != BL - 1:
                        for ch in range(NCH):
                            emit_out_ch(s, ch)
                    # else: defer out(s) into the final iteration as filler
                else:
                    emit_out_ch(s - 1, 0)
                    emit_wa(s, tail=True)
                    emit_out_ch(s - 1, 1)
                    emit_out_ch(s - 1, 2)
                    for ch in range(NCH):
                        emit_out_ch(s, ch, tail=True)

    nc.finalize()
    return nc


def _get_program():
    global _PROGRAM
    if _PROGRAM is None:
        _PROGRAM = _build_program()
    return _PROGRAM


def _q8(a, scale):
    return np.asarray(a.astype(np.float32) * np.float32(scale)).astype(FP8NP)


def _prep_inputs(x, x_h, Wg, bg, Wt, bt, Wp, bp, Ww, bw, gamma, beta,
                 run_mean, run_var):
    f32 = np.float32
    inv = (gamma / np.sqrt(run_var + 1e-5)).astype(f32)
    off = ((bw - run_mean) * inv + beta).astype(f32)

    xr = np.ascontiguousarray(x.reshape(B, CB, P, N), dtype=f32)
    xhr = np.ascontiguousarray(x_h.reshape(B, CB, P, N), dtype=f32)

    wt_eff = np.ascontiguousarray(Wt.T).astype(f32) / f32(DIM)   # [C, O]
    wp_eff = np.ascontiguousarray(Wp.T).astype(f32)
    wg_eff = np.ascontiguousarray(Wg.T).astype(f32)
    ww_eff = np.ascontiguousarray(Ww.T * inv[None, :]).astype(f32)

    x0 = xr[0].reshape(DIM, N)
    xh0 = xhr[0].reshape(DIM, N)
    th0 = wt_eff.T @ x0 + (bt.astype(f32) / f32(DIM))[:, None]
    ph0 = wp_eff.T @ xh0 + bp.astype(f32)[:, None]
    g0 = wg_eff.T @ xh0 + bg.astype(f32)[:, None]
    at0 = th0 @ ph0.T                   # att[i, j]
    wa0 = at0.T @ ww_eff                # WaT[j, o]
    MARG = f32(1.45)

    def s_of(a, marg=MARG):
        return f32(FP8TGT / (np.abs(a).max() * marg))

    s_x = s_of(xr, f32(1.0))
    s_xh = s_of(xhr, f32(1.0))
    s_wt = s_of(wt_eff, f32(1.0))
    s_wp = s_of(wp_eff, f32(1.0))
    s_wg = s_of(wg_eff, f32(1.0))
    s_ww = s_of(ww_eff, f32(1.0))
    s_th = s_of(th0)
    s_ph = s_of(ph0)
    s_g = s_of(g0)
    s_at = s_of(at0)
    s_wa = s_of(wa0)

    wstack = np.stack([
        _q8(wt_eff.reshape(CB, P, DIM), s_wt),
        _q8(wp_eff.reshape(CB, P, DIM), s_wp),
        _q8(wg_eff.reshape(CB, P, DIM), s_wg),
        _q8(ww_eff.reshape(CB, P, DIM), s_ww),
    ])                                          # [4, CB, P, DIM]
    wall = np.ascontiguousarray(wstack.transpose(2, 0, 1, 3))  # [P, 4, CB, DIM]

    brow = np.zeros((1, 2, DIM), dtype=BF16NP)
    brow[0, 0, :] = (bt.astype(f32) * (s_th / f32(DIM))).astype(BF16NP)
    brow[0, 1, :] = (bp.astype(f32) * s_ph).astype(BF16NP)

    ccols = np.zeros((P, 16), dtype=f32)
    ccols[:, 0:4] = bg.astype(f32).reshape(CB, P).T * f32(s_g)
    ccols[:, 4] = s_th / (s_x * s_wt)      # c_theta
    ccols[:, 5] = s_ph / (s_xh * s_wp)     # c_phi
    ccols[:, 6] = s_at / (s_th * s_ph)     # c_att
    ccols[:, 7] = s_g / (s_xh * s_wg)      # c_g
    ccols[:, 8] = s_wa / (s_at * s_ww)     # c_wa
    ccols[:, 9] = f32(1.0) / (s_wa * s_g)  # c_out

    shared = dict(wall=wall, brow=brow, ccols=ccols)

    def pmajor(a):
        # [BL, CB, P, N] -> [BL, P, NCH, CB, CHW] (chunk-major)
        a = a.reshape(a.shape[0], CB, a.shape[2], NCH, CHW)
        return np.ascontiguousarray(a.transpose(0, 2, 3, 1, 4))

    in_maps = []
    for k in range(NCORES):
        m = dict(shared)
        sl = slice(k * BL, (k + 1) * BL)
        m["xf"] = pmajor(xr[sl] + off.reshape(1, CB, P, 1)).astype(BF16NP)
        xq = _q8(xr[sl], s_x).reshape(BL, CB, P, NCH, CHW)
        xhq = _q8(xhr[sl], s_xh).reshape(BL, CB, P, NCH, CHW)
        xi = np.stack([xq, xhq], axis=1)        # [BL, 2, CB, P, NCH, CHW]
        m["xin"] = np.ascontiguousarray(xi.transpose(0, 3, 4, 1, 2, 5))
        in_maps.append(m)
    return in_maps


def run(inputs, trace=False, tmpdir=None):
    nc = _get_program()
    in_maps = _prep_inputs(**inputs)
    res = bass_utils.run_bass_kernel_spmd(
        nc, in_maps, core_ids=list(range(NCORES)), trace=trace, tmpdir=tmpdir)
    outs = [np.asarray(r["out4"]).astype(np.float32) for r in res.results]
    out = np.concatenate(outs, axis=0).transpose(0, 3, 1, 2, 4)  # [B,CB,P,NCH,CHW]
    out = np.ascontiguousarray(out).reshape(B, DIM, H, W)
    return out, res


def kernel(**inputs) -> np.ndarray:
    out, _ = run(inputs)
    return out


# revision 26
# speedup vs baseline: 1.0967x; 1.0001x over previous
"""Trainium2 Bass kernel for the sparse_attention (channel-attention) module.

Computation per sample (x_s, xh_s are [512, 1152] slices):
    theta = Wt @ x_s  + bt        (fold 1/512 into Wt, bt)
    phi   = Wp @ xh_s + bp
    g     = Wg @ xh_s + bg
    att   = theta @ phi^T         (contract over n; includes the /512)
    Wa    = (Ww*inv) @ att        (BN scale folded into Ww; 512^3 GEMM,
                                   cheaper than y = att @ g at 512^2*1152)
    out   = Wa @ g + xf           (xf = x + BN offset, bf16, host-folded)

Sharding: pure data parallel, 4 samples per core across 8 cores.

All GEMMs run in fp8 (e4m3, max 240) with DoubleRow perf mode. Per-tensor
scales come from a host sample-0 forward estimate with margin, so the
compiled program is data-independent. theta/phi are produced transposed
(thetaT[n,i]); att is produced as att[i,j] (stationary thetaT) which is the
stationary operand Wa needs, and WaT[j,o] is the stationary operand of the
out matmuls.

Schedule notes (from trace analysis):
- DMA engines are shared round-robin across ACTIVE rings, so bulk prefetch
  must queue BEHIND head-critical transfers. Inputs ride the sync HW ring
  in need-order (then bulk xf/next-sample FIFO behind); weights + out
  writes ride the scalar HW ring. The slow-start gpsimd SW ring is unused.
- PSUM: tag "pd" (DVE-evicted: theta/phi/out) and "pa" (ACT: g/att/Wa) get
  2 two-bank slots each so slot reuse always waits on the matching engine,
  with >=1.7us reuse distance in the emission order below.
- out evictions alternate DVE / Pool(gpsimd) so the out phase is not
  DVE-serial-bound; Wa sits between theta/phi p3 and p4 of the next
  sample's thpg so its ACT evictions hide under DVE-evicted fills.
"""

import numpy as np
import ml_dtypes

import concourse.bass as bass
import concourse.mybir as mybir
from concourse import bacc
from concourse.tile import TileContext
from concourse import bass_utils

B, DIM, H, W = 32, 512, 48, 24
N = H * W            # 1152
P = 128
CB = DIM // P        # 4 channel blocks
NB = N // P          # 9 n blocks
NCH = 3
CHW = N // NCH       # 384
NCORES = 8
BL = B // NCORES     # 4 samples per core

_f32 = mybir.dt.float32
_bf16 = mybir.dt.bfloat16
_fp8 = mybir.dt.float8e4
_add = mybir.AluOpType.add
_mult = mybir.AluOpType.mult
_DR = mybir.MatmulPerfMode.DoubleRow
_IDENT = mybir.ActivationFunctionType.Identity

FP8NP = ml_dtypes.float8_e4m3
BF16NP = ml_dtypes.bfloat16
FP8TGT = 192.0                      # of 240 max: saturation headroom

_PROGRAM = None


def _build_program():
    nc = bacc.Bacc("TRN2", target_bir_lowering=False, debug=False)

    # x and x_h fp8, chunk-interleaved: xin[s][:, c, 0] = x chunk c,
    # xin[s][:, c, 1] = x_h chunk c.
    xin = nc.dram_tensor("xin", [BL, P, NCH, 2, CB, CHW], _fp8,
                         kind="ExternalInput").ap()
    xf = nc.dram_tensor("xf", [BL, P, NCH, CB, CHW], _bf16,
                        kind="ExternalInput").ap()
    wall = nc.dram_tensor("wall", [P, 4, CB, DIM], _fp8,
                          kind="ExternalInput").ap()
    # bias rows (pre-scaled to consumer fp8 grid): a single-partition 2KB
    # tensor, broadcast across partitions on-device via a 1-contraction
    # matmul (saves 0.5MB of head-critical DMA)
    brow = nc.dram_tensor("brow", [1, 2, DIM], _bf16,
                          kind="ExternalInput").ap()
    # per-partition columns: bg*s_g per o-block (0:4), eviction scales (4:10)
    ccols = nc.dram_tensor("ccols", [P, 16], _f32, kind="ExternalInput").ap()
    out4 = nc.dram_tensor("out4", [BL, P, NCH, CB, CHW], _bf16,
                          kind="ExternalOutput").ap()

    with TileContext(nc) as tc:
        with tc.tile_pool(name="const", bufs=1) as cpool, \
             tc.tile_pool(name="xin", bufs=3) as xpool, \
             tc.tile_pool(name="xfin", bufs=3) as xfpool, \
             tc.tile_pool(name="work", bufs=6) as wpool, \
             tc.tile_pool(name="attwa", bufs=4) as apool, \
             tc.tile_pool(name="out", bufs=2) as opool, \
             tc.tile_pool(name="psum", bufs=2, space="PSUM") as psum:

            ccols_sb = cpool.tile([P, 16], _f32, tag="ccols")
            cb2 = cpool.tile([P, 2, 2, DIM], _bf16, tag="cbias")
            row_sb = cpool.tile([P, 2, DIM], _bf16, tag="brow")
            ones_sb = cpool.tile([P, P], _bf16, tag="ones")
            w_sb = cpool.tile([P, 4, CB, DIM], _fp8, tag="wall")
            wt_sb = w_sb[:, 0]
            wp_sb = w_sb[:, 1]
            wg_sb = w_sb[:, 2]
            ww_sb = w_sb[:, 3]

            btb2 = cb2[:, 0]               # [P, 2, DIM]
            btb = cb2[:, 0, 0]             # [P, DIM]
            bpb2 = cb2[:, 1]
            bpb = cb2[:, 1, 0]
            bgc = [ccols_sb[:, i:i + 1] for i in range(0, 4)]
            c_theta = ccols_sb[:, 4:5]
            c_phi = ccols_sb[:, 5:6]
            c_att = ccols_sb[:, 6:7]
            c_g = ccols_sb[:, 7:8]
            c_wa = ccols_sb[:, 8:9]
            c_out = ccols_sb[:, 9:10]

            st = [dict() for _ in range(BL)]

            def pd_tile():
                return psum.tile([P, 2, DIM], _f32, tag="pd", name="pd")

            def pa_tile():
                return psum.tile([P, 2, DIM], _f32, tag="pa", name="pa")

            def emit_warmup(n, nbias):
                """Dummy matmuls on a zeroed tile: keeps the PE continuously
                busy from the preamble until real data lands, so the p-state
                ramp (0.65 -> 1.2 -> 2.4GHz after 3us continuous) completes
                before the first real matmul. After `nbias` dummies, the PE
                broadcasts the bias rows (partition 0 of row_sb) across all
                partitions into a psum tile; ACT copies them into the
                duplicated SBUF layout the paired theta/phi evictions use."""
                zt = cpool.tile([P, 2, CHW], _fp8, tag="warm")
                nc.vector.memset(zt, 0)
                nc.vector.memset(ones_sb, 1.0)
                wps = pa_tile()
                bps = pd_tile()
                for i in range(n):
                    nc.tensor.matmul(wps[:, i % 2, :CHW], zt[:, :, 0:P], zt,
                                     start=True, stop=True, perf_mode=_DR)
                    if i == nbias:
                        for j in range(2):
                            nc.tensor.matmul(bps[:, j], ones_sb[0:1],
                                             row_sb[0:1, j],
                                             start=True, stop=True)
                        nc.scalar.activation(cb2[:, :, 0], bps, _IDENT,
                                             bias=0.0, scale=1.0)
                        nc.scalar.activation(cb2[:, :, 1], bps, _IDENT,
                                             bias=0.0, scale=1.0)

            def dma_head():
                """Both HW rings, need-order; bulk FIFOs behind."""
                x_sb = xpool.tile([P, NCH, 2, CB, CHW], _fp8, tag="xin",
                                  name="x_sb")
                xf_sb = xfpool.tile([P, NCH, CB, CHW], _bf16, tag="xf",
                                    name="xf_sb")
                st[0].update(x_sb=x_sb, xf_sb=xf_sb)
                x1 = xpool.tile([P, NCH, 2, CB, CHW], _fp8, tag="xin",
                                name="x_sb")
                xf1 = xfpool.tile([P, NCH, CB, CHW], _bf16, tag="xf",
                                  name="xf_sb")
                st[1].update(x_sb=x1, xf_sb=xf1)
                # sync HW ring (fast): consts + wt, then x/xh halves in use
                # order, then the bulk residuals FIFO behind
                nc.sync.dma_start(ccols_sb, ccols)
                nc.sync.dma_start(row_sb[0:1], brow)
                nc.sync.dma_start(w_sb[:, 0], wall[:, 0])
                nc.sync.dma_start(x_sb[:, 0, 0], xin[0][:, 0, 0])
                nc.sync.dma_start(x_sb[:, 0, 1], xin[0][:, 0, 1])
                nc.sync.dma_start(x_sb[:, 1, 0], xin[0][:, 1, 0])
                nc.sync.dma_start(x_sb[:, 1, 1], xin[0][:, 1, 1])
                nc.sync.dma_start(x_sb[:, 2, 0], xin[0][:, 2, 0])
                nc.sync.dma_start(x_sb[:, 2, 1], xin[0][:, 2, 1])
                nc.sync.dma_start(xf_sb, xf[0])
                nc.sync.dma_start(x1, xin[1])
                nc.sync.dma_start(xf1, xf[1])
                # scalar ring (parallel): wp+wg as one fat-line (4KB)
                # transfer, then ww (needed only ~15us in)
                nc.scalar.dma_start(w_sb[:, 1:3], wall[:, 1:3])
                nc.scalar.dma_start(w_sb[:, 3], wall[:, 3])

            def dma_sample(s):
                x_sb = xpool.tile([P, NCH, 2, CB, CHW], _fp8, tag="xin",
                                  name="x_sb")
                xf_sb = xfpool.tile([P, NCH, CB, CHW], _bf16, tag="xf",
                                    name="xf_sb")
                nc.sync.dma_start(x_sb, xin[s])
                nc.sync.dma_start(xf_sb, xf[s])
                st[s].update(x_sb=x_sb, xf_sb=xf_sb)

            def alloc_work(s):
                d = st[s]
                d["thetaT"] = wpool.tile([P, NB, DIM], _fp8, tag="work",
                                         name="thetaT")
                d["phiT"] = wpool.tile([P, NB, DIM], _fp8, tag="work",
                                       name="phiT")
                d["g_sb"] = wpool.tile([P, NCH, CB, CHW], _fp8, tag="work",
                                       name="g_sb")

            def conv_pair(x_sb, half, w, nbs):
                ps2 = pd_tile()
                for j, nb in enumerate(nbs):
                    c, jj = divmod(nb, NCH)
                    for k in range(CB // 2):
                        nc.tensor.matmul(
                            ps2[:, j],
                            x_sb[:, c, half, 2 * k:2 * k + 2,
                                 jj * P:(jj + 1) * P],
                            w[:, 2 * k:2 * k + 2],
                            start=(k == 0), stop=(k == CB // 2 - 1),
                            perf_mode=_DR)
                return ps2

            def emit_theta(s, p):
                d = st[s]
                nbs = [2 * p, 2 * p + 1] if 2 * p + 1 < NB else [NB - 1]
                ps2 = conv_pair(d["x_sb"], 0, wt_sb, nbs)
                if len(nbs) == 2:
                    nc.vector.scalar_tensor_tensor(
                        d["thetaT"][:, 2 * p:2 * p + 2], ps2, c_theta, btb2,
                        _mult, _add)
                else:
                    nc.vector.scalar_tensor_tensor(
                        d["thetaT"][:, NB - 1], ps2[:, 0], c_theta, btb,
                        _mult, _add)

            def emit_phi(s, p):
                d = st[s]
                nbs = [2 * p, 2 * p + 1] if 2 * p + 1 < NB else [NB - 1]
                ps2 = conv_pair(d["x_sb"], 1, wp_sb, nbs)
                if len(nbs) == 2:
                    nc.vector.scalar_tensor_tensor(
                        d["phiT"][:, 2 * p:2 * p + 2], ps2, c_phi, bpb2,
                        _mult, _add)
                else:
                    nc.vector.scalar_tensor_tensor(
                        d["phiT"][:, NB - 1], ps2[:, 0], c_phi, bpb,
                        _mult, _add)

            def emit_g(s, ob, chs):
                d = st[s]
                ps2 = pa_tile()
                for j, ch in enumerate(chs):
                    for k in range(CB // 2):
                        nc.tensor.matmul(
                            ps2[:, j, :CHW],
                            wg_sb[:, 2 * k:2 * k + 2, ob * P:(ob + 1) * P],
                            d["x_sb"][:, ch, 1, 2 * k:2 * k + 2, :],
                            start=(k == 0), stop=(k == CB // 2 - 1),
                            perf_mode=_DR)
                if len(chs) == 2:
                    nc.scalar.activation(
                        d["g_sb"][:, 0:2, ob], ps2[:, :, :CHW], _IDENT,
                        bias=bgc[ob], scale=c_g)
                else:
                    nc.scalar.activation(
                        d["g_sb"][:, chs[0], ob], ps2[:, 0, :CHW], _IDENT,
                        bias=bgc[ob], scale=c_g)

            def emit_thpg_A(s):
                alloc_work(s)
                for p in range(3):
                    emit_theta(s, p)
                    emit_phi(s, p)
                    emit_g(s, p, [0, 1])
                    emit_g(s, p, [2])

            def emit_att(s, tail=False):
                """att[i,j] = c_att * thetaT^T @ phiT (stationary thetaT)."""
                d = st[s]
                thetaT, phiT = d["thetaT"], d["phiT"]
                att_sb = apool.tile([P, CB, DIM], _fp8, tag="attwa",
                                    name="att_sb")
                d["att_sb"] = att_sb
                for p in range(CB // 2):
                    ps2 = pa_tile()
                    for j in range(2):
                        ib = 2 * p + j
                        for k in range(NB // 2):
                            nc.tensor.matmul(
                                ps2[:, j],
                                thetaT[:, 2 * k:2 * k + 2,
                                       ib * P:(ib + 1) * P],
                                phiT[:, 2 * k:2 * k + 2],
                                start=(k == 0), stop=False, perf_mode=_DR)
                        nc.tensor.matmul(
                            ps2[:, j], thetaT[:, NB - 1, ib * P:(ib + 1) * P],
                            phiT[:, NB - 1], start=False, stop=True)
                        if tail:
                            if j == 0:
                                nc.vector.tensor_scalar_mul(
                                    att_sb[:, 2 * p], ps2[:, 0], c_att)
                            else:
                                nc.scalar.activation(
                                    att_sb[:, 2 * p + 1], ps2[:, 1], _IDENT,
                                    bias=0.0, scale=c_att)
                    if not tail:
                        nc.scalar.activation(att_sb[:, 2 * p:2 * p + 2], ps2,
                                             _IDENT, bias=0.0, scale=c_att)

            def emit_wa(s, tail=False):
                """WaT[j,o] = c_wa * att^T @ (Ww*inv) (stationary att)."""
                d = st[s]
                att_sb = d["att_sb"]
                wa_sb = apool.tile([P, CB, DIM], _fp8, tag="attwa",
                                   name="wa_sb")
                d["wa_sb"] = wa_sb
                for p in range(CB // 2):
                    ps2 = pa_tile()
                    for j in range(2):
                        jb = 2 * p + j
                        for k in range(CB // 2):
                            nc.tensor.matmul(
                                ps2[:, j],
                                att_sb[:, 2 * k:2 * k + 2,
                                       jb * P:(jb + 1) * P],
                                ww_sb[:, 2 * k:2 * k + 2],
                                start=(k == 0), stop=(k == CB // 2 - 1),
                                perf_mode=_DR)
                        if tail:
                            if j == 0:
                                nc.vector.tensor_scalar_mul(
                                    wa_sb[:, 2 * p], ps2[:, 0], c_wa)
                            else:
                                nc.scalar.activation(
                                    wa_sb[:, 2 * p + 1], ps2[:, 1], _IDENT,
                                    bias=0.0, scale=c_wa)
                    if not tail:
                        nc.scalar.activation(wa_sb[:, 2 * p:2 * p + 2], ps2,
                                             _IDENT, bias=0.0, scale=c_wa)

            def emit_out_ch(s, ch, tail=False):
                """out[o,n] = c_out * WaT^T @ g + xf for one chunk.

                Tiles alternate between two drain paths so neither engine
                nor psum slot-set serializes: (a) DVE STT straight from
                PSUM into the pd slots; (b) ACT scale (frees the pa slot)
                + Pool in-place xf add (Pool cannot read PSUM)."""
                d = st[s]
                wa_sb, g_sb, xf_sb = d["wa_sb"], d["g_sb"], d["xf_sb"]
                o_sb = d["o_sb"]
                for p in range(CB // 2):
                    if tail:
                        # tail: Pool adds only on early tiles so the final
                        # evict->DMA chain is a single fast DVE STT
                        act_pool = (p == 0 and ch < 2)
                    elif s == BL - 2:
                        # deferred out (runs inside the final iteration):
                        # all-DVE, keeping Pool free for the tail tiles
                        act_pool = False
                    else:
                        act_pool = (ch + p) % 2 == 1
                    ps2 = pa_tile() if act_pool else pd_tile()
                    for j in range(2):
                        ob = 2 * p + j
                        for k in range(CB // 2):
                            nc.tensor.matmul(
                                ps2[:, j, :CHW],
                                wa_sb[:, 2 * k:2 * k + 2,
                                      ob * P:(ob + 1) * P],
                                g_sb[:, ch, 2 * k:2 * k + 2, :],
                                start=(k == 0), stop=(k == CB // 2 - 1),
                                perf_mode=_DR)
                    osl = o_sb[:, ch, 2 * p:2 * p + 2]
                    xsl = xf_sb[:, ch, 2 * p:2 * p + 2]
                    if act_pool:
                        nc.scalar.activation(osl, ps2[:, :, :CHW], _IDENT,
                                             bias=0.0, scale=c_out)
                        nc.gpsimd.tensor_add(osl, osl, xsl)
                    else:
                        nc.vector.scalar_tensor_tensor(
                            osl, ps2[:, :, :CHW], c_out, xsl, _mult, _add)
                    if tail:
                        q = nc.sync if (2 * ch + p) % 2 == 0 else nc.scalar
                        q.dma_start(out4[s][:, ch, 2 * p:2 * p + 2], osl)
                if not tail:
                    nc.sync.dma_start(out4[s][:, ch], o_sb[:, ch])

            def alloc_out(s):
                st[s]["o_sb"] = opool.tile([P, NCH, CB, CHW], _bf16,
                                           tag="osb", name="o_sb")

            # ---- schedule ----
            dma_head()
            emit_warmup(12, 4)
            emit_thpg_A(0)
            emit_theta(0, 3)
            emit_phi(0, 3)
            emit_theta(0, 4)
            emit_phi(0, 4)
            emit_g(0, 3, [0, 1])
            emit_g(0, 3, [2])
            for s in range(BL):
                if s + 2 < BL:
                    dma_sample(s + 2)
                last = (s == BL - 1)
                emit_att(s, tail=last)
                alloc_out(s)
                if not last:
                    emit_thpg_A(s + 1)
                    emit_theta(s + 1, 3)
                    emit_phi(s + 1, 3)
                    emit_wa(s)
                    emit_theta(s + 1, 4)
                    emit_phi(s + 1, 4)
                    emit_g(s + 1, 3, [0, 1])
                    emit_g(s + 1, 3, [2])
                    if s + 1 != BL - 1:
                        for ch in range(NCH):
                            emit_out_ch(s, ch)
                    # else: defer out(s) into the final iteration as filler
                else:
                    emit_out_ch(s - 1, 0)
                    emit_wa(s, tail=True)
                    emit_out_ch(s - 1, 1)
                    emit_out_ch(s - 1, 2)
                    for ch in range(NCH):
                        emit_out_ch(s, ch, tail=True)

    nc.finalize()
    return nc


def _get_program():
    global _PROGRAM
    if _PROGRAM is None:
        _PROGRAM = _build_program()
    return _PROGRAM


def _q8(a, scale):
    return np.asarray(a.astype(np.float32) * np.float32(scale)).astype(FP8NP)


def _prep_inputs(x, x_h, Wg, bg, Wt, bt, Wp, bp, Ww, bw, gamma, beta,
                 run_mean, run_var):
    f32 = np.float32
    inv = (gamma / np.sqrt(run_var + 1e-5)).astype(f32)
    off = ((bw - run_mean) * inv + beta).astype(f32)

    xr = np.ascontiguousarray(x.reshape(B, CB, P, N), dtype=f32)
    xhr = np.ascontiguousarray(x_h.reshape(B, CB, P, N), dtype=f32)

    wt_eff = np.ascontiguousarray(Wt.T).astype(f32) / f32(DIM)   # [C, O]
    wp_eff = np.ascontiguousarray(Wp.T).astype(f32)
    wg_eff = np.ascontiguousarray(Wg.T).astype(f32)
    ww_eff = np.ascontiguousarray(Ww.T * inv[None, :]).astype(f32)

    x0 = xr[0].reshape(DIM, N)
    xh0 = xhr[0].reshape(DIM, N)
    th0 = wt_eff.T @ x0 + (bt.astype(f32) / f32(DIM))[:, None]
    ph0 = wp_eff.T @ xh0 + bp.astype(f32)[:, None]
    g0 = wg_eff.T @ xh0 + bg.astype(f32)[:, None]
    at0 = th0 @ ph0.T                   # att[i, j]
    wa0 = at0.T @ ww_eff                # WaT[j, o]
    MARG = f32(1.45)

    def s_of(a, marg=MARG):
        return f32(FP8TGT / (np.abs(a).max() * marg))

    s_x = s_of(xr, f32(1.0))
    s_xh = s_of(xhr, f32(1.0))
    s_wt = s_of(wt_eff, f32(1.0))
    s_wp = s_of(wp_eff, f32(1.0))
    s_wg = s_of(wg_eff, f32(1.0))
    s_ww = s_of(ww_eff, f32(1.0))
    s_th = s_of(th0)
    s_ph = s_of(ph0)
    s_g = s_of(g0)
    s_at = s_of(at0)
    s_wa = s_of(wa0)

    wstack = np.stack([
        _q8(wt_eff.reshape(CB, P, DIM), s_wt),
        _q8(wp_eff.reshape(CB, P, DIM), s_wp),
        _q8(wg_eff.reshape(CB, P, DIM), s_wg),
        _q8(ww_eff.reshape(CB, P, DIM), s_ww),
    ])                                          # [4, CB, P, DIM]
    wall = np.ascontiguousarray(wstack.transpose(2, 0, 1, 3))  # [P, 4, CB, DIM]

    brow = np.zeros((1, 2, DIM), dtype=BF16NP)
    brow[0, 0, :] = (bt.astype(f32) * (s_th / f32(DIM))).astype(BF16NP)
    brow[0, 1, :] = (bp.astype(f32) * s_ph).astype(BF16NP)

    ccols = np.zeros((P, 16), dtype=f32)
    ccols[:, 0:4] = bg.astype(f32).reshape(CB, P).T * f32(s_g)
    ccols[:, 4] = s_th / (s_x * s_wt)      # c_theta
    ccols[:, 5] = s_ph / (s_xh * s_wp)     # c_phi
    ccols[:, 6] = s_at / (s_th * s_ph)     # c_att
    ccols[:, 7] = s_g / (s_xh * s_wg)      # c_g
    ccols[:, 8] = s_wa / (s_at * s_ww)     # c_wa
    ccols[:, 9] = f32(1.0) / (s_wa * s_g)  # c_out

    shared = dict(wall=wall, brow=brow, ccols=ccols)

    def pmajor(a):
        # [BL, CB, P, N] -> [BL, P, NCH, CB, CHW] (chunk-major)
        a = a.reshape(a.shape[0], CB, a.shape[2], NCH, CHW)
        return np.ascontiguousarray(a.transpose(0, 2, 3, 1, 4))

    in_maps = []
    for k in range(NCORES):
        m = dict(shared)
        sl = slice(k * BL, (k + 1) * BL)
        m["xf"] = pmajor(xr[sl] + off.reshape(1, CB, P, 1)).astype(BF16NP)
        xq = _q8(xr[sl], s_x).reshape(BL, CB, P, NCH, CHW)
        xhq = _q8(xhr[sl], s_xh).reshape(BL, CB, P, NCH, CHW)
        xi = np.stack([xq, xhq], axis=1)        # [BL, 2, CB, P, NCH, CHW]
        m["xin"] = np.ascontiguousarray(xi.transpose(0, 3, 4, 1, 2, 5))
        in_maps.append(m)
    return in_maps


def run(inputs, trace=False, tmpdir=None):
    nc = _get_program()
    in_maps = _prep_inputs(**inputs)
    res = bass_utils.run_bass_kernel_spmd(
        nc, in_maps, core_ids=list(range(NCORES)), trace=trace, tmpdir=tmpdir)
    outs = [np.asarray(r["out4"]).astype(np.float32) for r in res.results]
    out = np.concatenate(outs, axis=0).transpose(0, 3, 1, 2, 4)  # [B,CB,P,NCH,CHW]
    out = np.ascontiguousarray(out).reshape(B, DIM, H, W)
    return out, res


def kernel(**inputs) -> np.ndarray:
    out, _ = run(inputs)
    return out
